# revision 3
# baseline (speedup 1.0000x reference)
"""MoE (top-2 of 32 experts, 512->512) on 8 NeuronCores, expert-parallel.

Strategy (full-I/O contract: kernel() receives full inputs, returns full output):
  - Host computes the small selector (softmax gate + top-k) in fp32 numpy,
    mirroring the reference op-for-op, and performs the "all-to-all dispatch":
    tokens are grouped by expert into capacity-padded batches (this is the
    host-side sharding step of the expert-parallel layout).
  - Experts are sharded 4-per-core across the 8 cores.  Each core runs a Bass
    kernel: for each of its experts, Y = relu(X_e @ We[e] + be[e]) * gate_w,
    with the per-token gate weight fused into the relu via the ScalarE
    activation (scale is per-partition; gate weights are >= 0 so
    w*relu(z) == relu(w*z)).
  - Host combines: out[token] = sum of its k slot rows (weighted on device).

The device kernel is compiled once per capacity C and cached.
"""

import numpy as np
from contextlib import ExitStack

B, NIN, NOUT, E, NCORES = 8192, 512, 512, 32, 8
EPC = E // NCORES  # experts per core
KCH = NIN // 128   # contraction chunks

# fp32 matmuls on the PE (4 cycles/row). float32r would be 4x faster but has
# reduced mantissa precision; flip only if measured error is acceptable.
USE_F32R = False

_CACHE = {}


def _build(C):
    """Build + compile the per-core Bass program for capacity C (tokens per
    expert, multiple of 128). Returns (nc, runner)."""
    import concourse.mybir as mybir
    import concourse.tile as tile
    from concourse import bacc

    nblk = C // 128
    mmdt = mybir.dt.float32r if USE_F32R else mybir.dt.float32

    nc = bacc.Bacc("TRN2", target_bir_lowering=False, debug=False,
                   num_devices=NCORES)
    xt = nc.dram_tensor("xt", [NIN, EPC * C], mmdt, kind="ExternalInput").ap()
    we = nc.dram_tensor("we", [EPC * NIN, NOUT], mmdt, kind="ExternalInput").ap()
    be = nc.dram_tensor("be", [EPC, NOUT], mybir.dt.float32,
                        kind="ExternalInput").ap()
    cw = nc.dram_tensor("cw", [EPC * C], mybir.dt.float32,
                        kind="ExternalInput").ap()
    out = nc.dram_tensor("out", [EPC * C, NOUT], mybir.dt.float32,
                         kind="ExternalOutput").ap()

    with tile.TileContext(nc) as tc, ExitStack() as ctx:
        wpool = ctx.enter_context(tc.tile_pool(name="w", bufs=1))
        xpool = ctx.enter_context(tc.tile_pool(name="x", bufs=1))
        spool = ctx.enter_context(tc.tile_pool(name="s", bufs=1))
        pspool = ctx.enter_context(tc.tile_pool(name="ps", bufs=4, space="PSUM"))
        opool = ctx.enter_context(tc.tile_pool(name="o", bufs=4))

        # Expert weights, SBUF-resident: chunk (i, kc) at free offset
        # (i*KCH+kc)*NOUT.
        w_sb = wpool.tile([128, EPC * KCH * NOUT], mmdt)
        for i in range(EPC):
            for kc in range(KCH):
                nc.sync.dma_start(
                    w_sb[:, (i * KCH + kc) * NOUT:(i * KCH + kc + 1) * NOUT],
                    we[i * NIN + kc * 128:i * NIN + (kc + 1) * 128, :])

        # Gathered tokens (transposed: features on partitions): chunk kc,
        # expert i at free offset kc*(EPC*C) + i*C.
        x_sb = xpool.tile([128, KCH * EPC * C], mmdt)
        for i in range(EPC):
            for kc in range(KCH):
                nc.sync.dma_start(
                    x_sb[:, kc * (EPC * C) + i * C:kc * (EPC * C) + (i + 1) * C],
                    xt[kc * 128:(kc + 1) * 128, i * C:(i + 1) * C])

        # Biases replicated to all 128 partitions (step-0 DMA broadcast).
        bias_sb = spool.tile([128, EPC * NOUT], mybir.dt.float32)
        for i in range(EPC):
            nc.sync.dma_start(bias_sb[:, i * NOUT:(i + 1) * NOUT],
                              be[i:i + 1, :].to_broadcast((128, NOUT)))

        # Per-token gate weights: block b's 128 values land in column b.
        cw_sb = spool.tile([128, EPC * nblk], mybir.dt.float32)
        nc.sync.dma_start(cw_sb[:], cw.rearrange("(b p) -> p b", p=128))

        for i in range(EPC):
            for t in range(nblk):
                ps = pspool.tile([128, NOUT], mybir.dt.float32)
                for kc in range(KCH):
                    nc.tensor.matmul(
                        ps[:],
                        lhsT=x_sb[:, kc * (EPC * C) + i * C + t * 128:
                                  kc * (EPC * C) + i * C + (t + 1) * 128],
                        rhs=w_sb[:, (i * KCH + kc) * NOUT:
                                 (i * KCH + kc + 1) * NOUT],
                        start=(kc == 0), stop=(kc == KCH - 1))
                nc.vector.tensor_add(ps[:], ps[:],
                                     bias_sb[:, i * NOUT:(i + 1) * NOUT])
                ot = opool.tile([128, NOUT], mybir.dt.float32)
                nc.scalar.activation(
                    ot[:], ps[:], mybir.ActivationFunctionType.Relu,
                    scale=cw_sb[:, i * nblk + t:i * nblk + t + 1])
                nc.sync.dma_start(out[i * C + t * 128:i * C + (t + 1) * 128, :],
                                  ot[:])

    nc.compile()
    return nc


def _make_runner(nc):
    """One-time jit of the 8-core SPMD executable (mirrors
    bass2jax.run_bass_via_pjrt, cached so repeat calls skip retracing)."""
    import jax
    import jax.core
    import numpy as _np
    from jax.sharding import Mesh, PartitionSpec
    from jax.experimental.shard_map import shard_map
    from concourse import bass2jax, mybir

    bass2jax.install_neuronx_cc_hook()

    partition_name = (nc.partition_id_tensor.name
                      if nc.partition_id_tensor else None)
    in_names, out_names, out_avals, zero_shapes = [], [], [], []
    for alloc in nc.m.functions[0].allocations:
        if not isinstance(alloc, mybir.MemoryLocationSet):
            continue
        name = alloc.memorylocations[0].name
        if alloc.kind == "ExternalInput":
            if name != partition_name:
                in_names.append(name)
        elif alloc.kind == "ExternalOutput":
            out_names.append(name)
            shape = tuple(alloc.tensor_shape)
            dt = mybir.dt.np(alloc.dtype)
            out_avals.append(jax.core.ShapedArray(shape, dt))
            zero_shapes.append((shape, dt))
    n_params = len(in_names)
    all_names = in_names + out_names
    if partition_name is not None:
        all_names = all_names + [partition_name]

    def _body(*args):
        operands = list(args)
        if partition_name is not None:
            operands.append(bass2jax.partition_id_tensor())
        outs = bass2jax._bass_exec_p.bind(
            *operands,
            out_avals=tuple(out_avals),
            in_names=tuple(all_names),
            out_names=tuple(out_names),
            lowering_input_output_aliases=(),
            sim_require_finite=True,
            sim_require_nnan=True,
            nc=nc,
        )
        return tuple(outs)

    devices = jax.devices()[:NCORES]
    mesh = Mesh(_np.asarray(devices), ("core",))
    n_outs = len(out_names)
    specs = (PartitionSpec("core"),) * (n_params + n_outs)
    donate = tuple(range(n_params, n_params + n_outs))
    sharded = jax.jit(
        shard_map(_body, mesh=mesh, in_specs=specs,
                  out_specs=(PartitionSpec("core"),) * n_outs,
                  check_rep=False),
        donate_argnums=donate, keep_unused=True)

    def run(in_maps):
        concat_in = [
            _np.concatenate([m[name] for m in in_maps], axis=0)
            for name in in_names
        ]
        concat_zeros = [
            _np.zeros((NCORES * s[0],) + tuple(s[1:]), dt)
            for (s, dt) in zero_shapes
        ]
        out_arrs = sharded(*concat_in, *concat_zeros)
        return [
            {name: _np.asarray(out_arrs[i]).reshape(
                (NCORES,) + tuple(out_avals[i].shape))[c]
             for i, name in enumerate(out_names)}
            for c in range(NCORES)
        ]

    # exposed for benchmarking (test.py)
    run._sharded = sharded
    run._in_names = in_names
    run._zero_shapes = zero_shapes
    return run


def _get_runner(C):
    if C not in _CACHE:
        nc = _build(C)
        _CACHE[C] = (nc, _make_runner(nc))
    return _CACHE[C]


def _route(x, Wg, bg, k):
    """Replicates the reference selector in fp32: softmax gate, top-k
    (stable, ties to lower index like jax.lax.top_k), aux loss."""
    logits = x @ Wg + bg
    m = logits.max(-1, keepdims=True)
    p = np.exp(logits - m)
    gate = p / p.sum(-1, keepdims=True)
    idx = np.argsort(-gate, axis=-1, kind="stable")[:, :k]      # [B, k]
    vals = np.take_along_axis(gate, idx, axis=-1)               # [B, k]
    row_sum = gate.sum(-1)
    aux = (np.var(row_sum) / (np.mean(row_sum) ** 2 + np.float32(1e-10)))
    return idx, vals, np.float32(aux)


def kernel(x, Wg, bg, We, be, k):
    x = np.ascontiguousarray(np.asarray(x, dtype=np.float32))
    Wg = np.asarray(Wg, dtype=np.float32)
    bg = np.asarray(bg, dtype=np.float32)
    We = np.ascontiguousarray(np.asarray(We, dtype=np.float32))
    be = np.ascontiguousarray(np.asarray(be, dtype=np.float32))
    k = int(k)

    idx, vals, aux = _route(x, Wg, bg, k)

    # Dispatch: flatten (token, slot) pairs, group by expert, capacity-pad.
    ef = idx.ravel()
    tf = np.repeat(np.arange(B), k)
    wf = vals.ravel()
    order = np.argsort(ef, kind="stable")
    e_sorted, t_sorted, w_sorted = ef[order], tf[order], wf[order]
    counts = np.bincount(ef, minlength=E)
    C = int(max(128, -(-counts.max() // 128) * 128))

    starts = np.zeros(E, dtype=np.int64)
    starts[1:] = np.cumsum(counts)[:-1]
    dest = e_sorted * C + (np.arange(B * k) - starts[e_sorted])  # unique rows

    Xg = np.zeros((E * C, NIN), dtype=np.float32)
    Xg[dest] = x[t_sorted]
    cwg = np.zeros(E * C, dtype=np.float32)
    cwg[dest] = w_sorted

    in_maps = []
    for c in range(NCORES):
        seg = Xg[c * EPC * C:(c + 1) * EPC * C]                 # [EPC*C, NIN]
        in_maps.append({
            "xt": np.ascontiguousarray(seg.T),                  # [NIN, EPC*C]
            "we": We[c * EPC:(c + 1) * EPC].reshape(EPC * NIN, NOUT),
            "be": be[c * EPC:(c + 1) * EPC],
            "cw": cwg[c * EPC * C:(c + 1) * EPC * C],
        })

    _, run = _get_runner(C)
    results = run(in_maps)

    Y = np.concatenate([results[c]["out"] for c in range(NCORES)], axis=0)
    slot_rows = Y[dest[np.argsort(order, kind="stable")]]       # token-major
    out = slot_rows.reshape(B, k, NOUT).sum(axis=1, dtype=np.float32)
    return out, aux


# revision 6
# speedup vs baseline: 1.2658x; 1.2658x over previous
"""MoE (top-2 of 32 experts, 512->512) on 8 NeuronCores, expert-parallel.

Strategy (full-I/O contract: kernel() receives full inputs, returns full output):
  - Host computes the small selector (softmax gate + top-k) in fp32 numpy,
    mirroring the reference op-for-op, and performs the "all-to-all dispatch":
    tokens are grouped by expert into capacity-padded batches (the host-side
    sharding step of the expert-parallel layout).
  - Experts are sharded 4-per-core across the 8 cores.  Each core runs a Bass
    kernel: for each of its experts, Y = relu(X_e @ We[e] + be[e]) * gate_w,
    with the per-token gate weight fused into the relu via the ScalarE
    activation (scale is per-partition; gate weights are >= 0 so
    w*relu(z) == relu(w*z)).
  - Host combines: out[token] = sum of its k slot rows (weighted on device).

Precision modes for the expert GEMMs (MODE):
  - "f32"  : exact fp32 matmuls (PE runs them as 2 half-rate passes).
  - "f16x3": x and W split into fp16 hi+lo; y = xh@wh + xh@wl + xl@wh.
             fp16 products are exact in fp32 (11+11 mantissa bits < 24), so
             the only extra error is the dropped xl@wl term (~2^-22 relative)
             - fp32-equivalent accuracy at 3/4 the PE cost of "f32".
  - "f32r" : single-pass relaxed-precision fp32 (tf32-like, ~1e-4 rel err).
  - "bf16" : single-pass bf16 (~1e-3 rel err).

All tensors are pre-swizzled on the host into the exact SBUF layout so every
device DMA is a contiguous copy.  The device kernel is compiled once per
(capacity, mode, has_bias) and cached.
"""

import numpy as np
from contextlib import ExitStack

B, NIN, NOUT, E, NCORES = 8192, 512, 512, 32, 8
EPC = E // NCORES  # experts per core
KCH = NIN // 128   # contraction chunks of 128

MODE = "f16x3"

_CACHE = {}


def _mode_spec(mode):
    import concourse.mybir as mybir
    import ml_dtypes
    if mode == "f32":
        return dict(dt=mybir.dt.float32, npdt=np.float32, nsplit=1,
                    passes=[(0, 0)])
    if mode == "f32r":
        return dict(dt=mybir.dt.float32r, npdt=np.float32, nsplit=1,
                    passes=[(0, 0)])
    if mode == "bf16":
        return dict(dt=mybir.dt.bfloat16, npdt=ml_dtypes.bfloat16, nsplit=1,
                    passes=[(0, 0)])
    if mode == "f16x3":
        return dict(dt=mybir.dt.float16, npdt=np.float16, nsplit=2,
                    passes=[(0, 0), (0, 1), (1, 0)])
    raise ValueError(mode)


def _build(C, mode, has_bias):
    """Build + compile the per-core Bass program for capacity C (tokens per
    expert, multiple of 128)."""
    import concourse.mybir as mybir
    import concourse.tile as tile
    from concourse import bacc

    spec = _mode_spec(mode)
    mmdt = spec["dt"]
    nsplit = spec["nsplit"]
    passes = spec["passes"]
    nblk = C // 128
    XF = EPC * KCH * C      # x free elems per split: [p][i][kc][c]
    WF = EPC * KCH * NOUT   # w free elems per split: [p][i][kc][n]

    nc = bacc.Bacc("TRN2", target_bir_lowering=False, debug=False,
                   num_devices=NCORES)
    xs = [nc.dram_tensor(f"x{s}", [128, XF], mmdt, kind="ExternalInput").ap()
          for s in range(nsplit)]
    ws = [nc.dram_tensor(f"w{s}", [128, WF], mmdt, kind="ExternalInput").ap()
          for s in range(nsplit)]
    cw = nc.dram_tensor("cw", [128, EPC * nblk], mybir.dt.float32,
                        kind="ExternalInput").ap()
    if has_bias:
        be = nc.dram_tensor("be", [EPC, NOUT], mybir.dt.float32,
                            kind="ExternalInput").ap()
    out = nc.dram_tensor("out", [EPC * C, NOUT], mybir.dt.float32,
                         kind="ExternalOutput").ap()

    with tile.TileContext(nc) as tc, ExitStack() as ctx:
        xpool = ctx.enter_context(tc.tile_pool(name="x", bufs=1))
        wpool = ctx.enter_context(tc.tile_pool(name="w", bufs=1))
        spool = ctx.enter_context(tc.tile_pool(name="s", bufs=1))
        opool = ctx.enter_context(tc.tile_pool(name="o", bufs=1))
        pspool = ctx.enter_context(tc.tile_pool(name="ps", bufs=4,
                                                space="PSUM"))

        x_sb = [xpool.tile([128, XF], mmdt, name=f"x{s}_sb", tag=f"x{s}")
                for s in range(nsplit)]
        w_sb = [wpool.tile([128, WF], mmdt, name=f"w{s}_sb", tag=f"w{s}")
                for s in range(nsplit)]
        # per-expert contiguous slices so compute can start after ~1/EPC of
        # the input DMA
        for i in range(EPC):
            for s in range(nsplit):
                nc.sync.dma_start(
                    w_sb[s][:, i * KCH * NOUT:(i + 1) * KCH * NOUT],
                    ws[s][:, i * KCH * NOUT:(i + 1) * KCH * NOUT])
                nc.sync.dma_start(
                    x_sb[s][:, i * KCH * C:(i + 1) * KCH * C],
                    xs[s][:, i * KCH * C:(i + 1) * KCH * C])

        cw_sb = spool.tile([128, EPC * nblk], mybir.dt.float32)
        nc.sync.dma_start(cw_sb[:], cw)

        if has_bias:
            bias_sb = spool.tile([EPC, NOUT], mybir.dt.float32)
            nc.sync.dma_start(bias_sb[:], be)
            ones_sb = spool.tile([1, 128], mybir.dt.float32)
            nc.vector.memset(ones_sb[:], 1.0)

        out_sb = opool.tile([128, EPC * nblk * NOUT], mybir.dt.float32)

        for i in range(EPC):
            for t in range(nblk):
                blk = i * nblk + t
                ps = pspool.tile([128, NOUT], mybir.dt.float32)
                nmm = KCH * len(passes) + (1 if has_bias else 0)
                m = 0
                for kc in range(KCH):
                    xoff = (i * KCH + kc) * C + t * 128
                    woff = (i * KCH + kc) * NOUT
                    for (sx, sw) in passes:
                        m += 1
                        nc.tensor.matmul(
                            ps[:],
                            lhsT=x_sb[sx][:, xoff:xoff + 128],
                            rhs=w_sb[sw][:, woff:woff + NOUT],
                            start=(m == 1), stop=(m == nmm))
                if has_bias:
                    nc.tensor.matmul(
                        ps[:], lhsT=ones_sb[:], rhs=bias_sb[i:i + 1, :],
                        start=False, stop=True)
                nc.scalar.activation(
                    out_sb[:, blk * NOUT:(blk + 1) * NOUT], ps[:],
                    mybir.ActivationFunctionType.Relu,
                    scale=cw_sb[:, blk:blk + 1])
                nc.sync.dma_start(
                    out[i * C + t * 128:i * C + (t + 1) * 128, :],
                    out_sb[:, blk * NOUT:(blk + 1) * NOUT])

    nc.compile()
    return nc


def _make_runner(nc):
    """One-time jit of the 8-core SPMD executable (mirrors
    bass2jax.run_bass_via_pjrt, cached so repeat calls skip retracing)."""
    import jax
    import jax.core
    import numpy as _np
    from jax.sharding import Mesh, PartitionSpec
    from jax.experimental.shard_map import shard_map
    from concourse import bass2jax, mybir

    bass2jax.install_neuronx_cc_hook()

    partition_name = (nc.partition_id_tensor.name
                      if nc.partition_id_tensor else None)
    in_names, out_names, out_avals, zero_shapes = [], [], [], []
    for alloc in nc.m.functions[0].allocations:
        if not isinstance(alloc, mybir.MemoryLocationSet):
            continue
        name = alloc.memorylocations[0].name
        if alloc.kind == "ExternalInput":
            if name != partition_name:
                in_names.append(name)
        elif alloc.kind == "ExternalOutput":
            out_names.append(name)
            shape = tuple(alloc.tensor_shape)
            dt = mybir.dt.np(alloc.dtype)
            out_avals.append(jax.core.ShapedArray(shape, dt))
            zero_shapes.append((shape, dt))
    n_params = len(in_names)
    all_names = in_names + out_names
    if partition_name is not None:
        all_names = all_names + [partition_name]

    def _body(*args):
        operands = list(args)
        if partition_name is not None:
            operands.append(bass2jax.partition_id_tensor())
        outs = bass2jax._bass_exec_p.bind(
            *operands,
            out_avals=tuple(out_avals),
            in_names=tuple(all_names),
            out_names=tuple(out_names),
            lowering_input_output_aliases=(),
            sim_require_finite=True,
            sim_require_nnan=True,
            nc=nc,
        )
        return tuple(outs)

    devices = jax.devices()[:NCORES]
    mesh = Mesh(_np.asarray(devices), ("core",))
    n_outs = len(out_names)
    specs = (PartitionSpec("core"),) * (n_params + n_outs)
    donate = tuple(range(n_params, n_params + n_outs))
    sharded = jax.jit(
        shard_map(_body, mesh=mesh, in_specs=specs,
                  out_specs=(PartitionSpec("core"),) * n_outs,
                  check_rep=False),
        donate_argnums=donate, keep_unused=True)

    def run(in_maps):
        concat_in = [
            _np.concatenate([m[name] for m in in_maps], axis=0)
            for name in in_names
        ]
        concat_zeros = [
            _np.zeros((NCORES * s[0],) + tuple(s[1:]), dt)
            for (s, dt) in zero_shapes
        ]
        out_arrs = sharded(*concat_in, *concat_zeros)
        return [
            {name: _np.asarray(out_arrs[i]).reshape(
                (NCORES,) + tuple(out_avals[i].shape))[c]
             for i, name in enumerate(out_names)}
            for c in range(NCORES)
        ]

    # exposed for benchmarking (test.py)
    run._sharded = sharded
    run._in_names = in_names
    run._zero_shapes = zero_shapes
    return run


def _get_runner(C, mode, has_bias):
    key = (C, mode, has_bias)
    if key not in _CACHE:
        nc = _build(C, mode, has_bias)
        _CACHE[key] = (nc, _make_runner(nc))
    return _CACHE[key]


def _route(x, Wg, bg, k):
    """Replicates the reference selector in fp32: softmax gate, top-k
    (stable, ties to lower index like jax.lax.top_k), aux loss."""
    logits = x @ Wg + bg
    m = logits.max(-1, keepdims=True)
    p = np.exp(logits - m)
    gate = p / p.sum(-1, keepdims=True)
    idx = np.argsort(-gate, axis=-1, kind="stable")[:, :k]      # [B, k]
    vals = np.take_along_axis(gate, idx, axis=-1)               # [B, k]
    row_sum = gate.sum(-1)
    aux = (np.var(row_sum) / (np.mean(row_sum) ** 2 + np.float32(1e-10)))
    return idx, vals, np.float32(aux)


def _split(arr, spec):
    """arr [NCORES, 128, F] fp32 -> list of nsplit arrays in device dtype."""
    if spec["nsplit"] == 1:
        return [np.ascontiguousarray(arr.astype(spec["npdt"]))]
    hi = arr.astype(np.float16)
    lo = (arr - hi.astype(np.float32)).astype(np.float16)
    return [np.ascontiguousarray(hi), np.ascontiguousarray(lo)]


def _prepare(x, Wg, bg, We, be, k, mode):
    """Route + dispatch: returns (C, in_maps, dest, inv, aux)."""
    spec = _mode_spec(mode)
    idx, vals, aux = _route(x, Wg, bg, k)

    ef = idx.ravel()
    wf = vals.ravel()
    tf = np.repeat(np.arange(B), k)
    order = np.argsort(ef, kind="stable")
    counts = np.bincount(ef, minlength=E)
    C = int(max(128, -(-counts.max() // 128) * 128))
    nblk = C // 128

    starts = np.zeros(E, dtype=np.int64)
    starts[1:] = np.cumsum(counts)[:-1]
    dest = ef[order] * C + (np.arange(B * k) - starts[ef[order]])  # unique
    inv = np.argsort(order, kind="stable")

    Xg = np.zeros((E * C, NIN), dtype=np.float32)
    Xg[dest] = x[tf[order]]
    cwg = np.zeros(E * C, dtype=np.float32)
    cwg[dest] = wf[order]

    # swizzle into SBUF layouts
    # x: [core][p][i][kc][c]
    xsw = Xg.reshape(NCORES, EPC, C, KCH, 128).transpose(0, 4, 1, 3, 2) \
        .reshape(NCORES, 128, EPC * KCH * C)
    xarrs = _split(xsw, spec)
    # w: [core][p][i][kc][n]
    wsw = We.reshape(NCORES, EPC, KCH, 128, NOUT).transpose(0, 3, 1, 2, 4) \
        .reshape(NCORES, 128, EPC * KCH * NOUT)
    warrs = _split(wsw, spec)
    # cw: [core][p][i][t]
    csw = np.ascontiguousarray(
        cwg.reshape(NCORES, EPC, nblk, 128).transpose(0, 3, 1, 2)
        .reshape(NCORES, 128, EPC * nblk))

    has_bias = bool(np.any(be))
    in_maps = []
    for c in range(NCORES):
        m = {"cw": csw[c]}
        for s in range(spec["nsplit"]):
            m[f"x{s}"] = xarrs[s][c]
            m[f"w{s}"] = warrs[s][c]
        if has_bias:
            m["be"] = np.ascontiguousarray(be[c * EPC:(c + 1) * EPC])
        in_maps.append(m)
    return C, has_bias, in_maps, dest, inv, aux


def kernel(x, Wg, bg, We, be, k):
    x = np.ascontiguousarray(np.asarray(x, dtype=np.float32))
    Wg = np.asarray(Wg, dtype=np.float32)
    bg = np.asarray(bg, dtype=np.float32)
    We = np.ascontiguousarray(np.asarray(We, dtype=np.float32))
    be = np.ascontiguousarray(np.asarray(be, dtype=np.float32))
    k = int(k)

    C, has_bias, in_maps, dest, inv, aux = _prepare(x, Wg, bg, We, be, k, MODE)
    _, run = _get_runner(C, MODE, has_bias)
    results = run(in_maps)

    Y = np.concatenate([results[c]["out"] for c in range(NCORES)], axis=0)
    slot_rows = Y[dest[inv]]                                    # token-major
    out = slot_rows.reshape(B, k, NOUT).sum(axis=1, dtype=np.float32)
    return out, aux


# revision 21
# speedup vs baseline: 1.6061x; 1.2688x over previous
"""MoE (top-2 of 32 experts, 512->512) on 8 NeuronCores, expert-parallel.

Strategy (full-I/O contract: kernel() receives full inputs, returns full output):
  - Host computes the small selector (softmax gate + top-k) in fp32 numpy,
    mirroring the reference op-for-op, and performs the "all-to-all dispatch":
    tokens are grouped by expert into capacity-padded batches (the host-side
    sharding step of the expert-parallel layout).
  - Experts are sharded 4-per-core across the 8 cores.  Each core runs a Bass
    kernel: for each of its experts, Y = relu(X_e @ We[e] + be[e]) * gate_w,
    with the per-token gate weight fused into the relu via the ScalarE
    activation (scale is per-partition; gate weights are >= 0 so
    w*relu(z) == relu(w*z)).
  - Host combines: out[token] = sum of its k slot rows (weighted on device).

Precision modes for the expert GEMMs (MODE):
  - "f32"  : exact fp32 matmuls (PE runs them as 2 half-rate passes).
  - "f16x3": x and W split into fp16 hi+lo; y = xh@wh + xh@wl + xl@wh.
             fp16 products are exact in fp32 (11+11 mantissa bits < 24), so
             the only extra error is the dropped xl@wl term (~2^-22 relative)
             - fp32-equivalent accuracy at 3/4 the PE cost of "f32".
  - "f32r" : single-pass relaxed-precision fp32 (tf32-like, ~1e-4 rel err).
  - "bf16" : single-pass bf16 (~1e-3 rel err).

All tensors are pre-swizzled on the host into the exact SBUF layout so every
device DMA is a contiguous copy.  The device kernel is compiled once per
(capacity, mode, has_bias) and cached.
"""

import numpy as np
from contextlib import ExitStack

B, NIN, NOUT, E, NCORES = 8192, 512, 512, 32, 8
EPC = E // NCORES  # experts per core
KCH = NIN // 128   # contraction chunks of 128

MODE = "f16x3"

_CACHE = {}


def _mode_spec(mode):
    import concourse.mybir as mybir
    import ml_dtypes
    if mode == "f32":
        return dict(dt=mybir.dt.float32, npdt=np.float32, nsplit=1,
                    passes=[(0, 0)])
    if mode == "f32r":
        return dict(dt=mybir.dt.float32r, npdt=np.float32, nsplit=1,
                    passes=[(0, 0)])
    if mode == "bf16":
        return dict(dt=mybir.dt.bfloat16, npdt=ml_dtypes.bfloat16, nsplit=1,
                    passes=[(0, 0)])
    if mode == "f16x3":
        return dict(dt=mybir.dt.float16, npdt=np.float16, nsplit=2,
                    passes=[(0, 0), (0, 1), (1, 0)])
    raise ValueError(mode)


def _build(C, mode, has_bias):
    """Build + compile the per-core Bass program for capacity C (tokens per
    expert, multiple of 128)."""
    import concourse.mybir as mybir
    import concourse.tile as tile
    from concourse import bacc

    spec = _mode_spec(mode)
    mmdt = spec["dt"]
    nsplit = spec["nsplit"]
    passes = spec["passes"]
    nblk = C // 128
    XF = EPC * KCH * C      # x free elems per split: [p][i][kc][c]
    WF = EPC * KCH * NOUT   # w free elems per split: [p][i][kc][n]

    nc = bacc.Bacc("TRN2", target_bir_lowering=False, debug=False,
                   num_devices=NCORES)
    # hi/lo splits are stacked along the free axis of ONE tensor so a single
    # DMA moves both (fewer DMA triggers - they serialize on the HWDGE ring)
    xd = nc.dram_tensor("xd", [128, nsplit * XF], mmdt,
                        kind="ExternalInput").ap()
    wd = nc.dram_tensor("wd", [128, nsplit * WF], mmdt,
                        kind="ExternalInput").ap()
    cw = nc.dram_tensor("cw", [128, EPC * nblk], mybir.dt.float32,
                        kind="ExternalInput").ap()
    if has_bias:
        be = nc.dram_tensor("be", [EPC, NOUT], mybir.dt.float32,
                            kind="ExternalInput").ap()
    out = nc.dram_tensor("out", [EPC * C, NOUT], mybir.dt.float32,
                         kind="ExternalOutput").ap()

    with tile.TileContext(nc) as tc, ExitStack() as ctx:
        xpool = ctx.enter_context(tc.tile_pool(name="x", bufs=1))
        wpool = ctx.enter_context(tc.tile_pool(name="w", bufs=1))
        spool = ctx.enter_context(tc.tile_pool(name="s", bufs=1))
        opool = ctx.enter_context(tc.tile_pool(name="o", bufs=1))
        pspool = ctx.enter_context(tc.tile_pool(name="ps", bufs=1,
                                                space="PSUM"))

        x_all = xpool.tile([128, nsplit * XF], mmdt, name="x_all")
        w_all = wpool.tile([128, nsplit * WF], mmdt, name="w_all")
        x_sb = [x_all[:, s * XF:(s + 1) * XF] for s in range(nsplit)]
        w_sb = [w_all[:, s * WF:(s + 1) * WF] for s in range(nsplit)]

        # cw (and bias) first: tiny, and every ACT epilogue needs it - behind
        # the bulk inputs on the FIFO ring it would stall all PSUM recycling.
        cw_sb = spool.tile([128, EPC * nblk], mybir.dt.float32)
        nc.sync.dma_start(cw_sb[:], cw)
        if has_bias:
            bias_sb = spool.tile([EPC, NOUT], mybir.dt.float32)
            nc.sync.dma_start(bias_sb[:], be)
            ones_sb = spool.tile([1, 128], mybir.dt.float32)
            nc.vector.memset(ones_sb[:], 1.0)

        # Input DMAs per (expert, kc-half): one DMA covers hi+lo (2 strided
        # regions).  Inputs ride the SP HWDGE ring; outputs ride the ACT ring
        # to avoid head-of-line blocking between the two streams.
        xv = xd.rearrange("p (s f) -> p s f", s=nsplit)
        wv = wd.rearrange("p (s f) -> p s f", s=nsplit)
        xav = x_all[:].rearrange("p (s f) -> p s f", s=nsplit)
        wav = w_all[:].rearrange("p (s f) -> p s f", s=nsplit)
        for i in range(EPC):
            # finer chunks for expert 0 so the first matmul starts sooner
            KH = 1 if i == 0 else 2
            for kh in range(0, KCH, KH):
                xl_ = (i * KCH + kh) * C
                xr_ = (i * KCH + kh + KH) * C
                wl_ = (i * KCH + kh) * NOUT
                wr_ = (i * KCH + kh + KH) * NOUT
                nc.sync.dma_start(xav[:, :, xl_:xr_], xv[:, :, xl_:xr_])
                nc.sync.dma_start(wav[:, :, wl_:wr_], wv[:, :, wl_:wr_])

        out_sb = opool.tile([128, EPC * nblk * NOUT], mybir.dt.float32)

        npass = len(passes)
        nmm = KCH * npass + (1 if has_bias else 0)
        for i in range(EPC):
            # k-chunk outer, token-block inner: each block's PSUM bank stays
            # live across the kc sweep, so the first matmuls only need the
            # first (x, w) k-chunk of this expert.
            pss = [pspool.tile([128, NOUT], mybir.dt.float32,
                               name=f"ps_{i}_{t}", tag=f"ps{t}")
                   for t in range(nblk)]
            for kc in range(KCH):
                xoff = (i * KCH + kc) * C
                woff = (i * KCH + kc) * NOUT
                for t in range(nblk):
                    for p, (sx, sw) in enumerate(passes):
                        m = kc * npass + p + 1
                        nc.tensor.matmul(
                            pss[t][:],
                            lhsT=x_sb[sx][:, xoff + t * 128:
                                          xoff + (t + 1) * 128],
                            rhs=w_sb[sw][:, woff:woff + NOUT],
                            start=(m == 1), stop=(m == nmm))
            for t in range(nblk):
                blk = i * nblk + t
                if has_bias:
                    nc.tensor.matmul(
                        pss[t][:], lhsT=ones_sb[:], rhs=bias_sb[i:i + 1, :],
                        start=False, stop=True)
                nc.scalar.activation(
                    out_sb[:, blk * NOUT:(blk + 1) * NOUT], pss[t][:],
                    mybir.ActivationFunctionType.Relu,
                    scale=cw_sb[:, blk:blk + 1])
            # out DMAs per expert-half (fewer triggers); per-block for the
            # last expert so the kernel tail is short
            if i < EPC - 1:
                tranges = ((0, nblk - nblk // 2), (nblk - nblk // 2, nblk))
            else:
                tranges = tuple((t, t + 1) for t in range(nblk))
            for (t0, t1) in tranges:
                nc.scalar.dma_start(
                    out[i * C + t0 * 128:i * C + t1 * 128, :]
                    .rearrange("(t p) n -> p t n", p=128),
                    out_sb[:].rearrange("p (b n) -> p b n", n=NOUT)
                    [:, i * nblk + t0:i * nblk + t1, :])

    nc.compile()
    return nc


def _make_runner(nc):
    """One-time jit of the 8-core SPMD executable (mirrors
    bass2jax.run_bass_via_pjrt, cached so repeat calls skip retracing)."""
    import jax
    import jax.core
    import numpy as _np
    from jax.sharding import Mesh, PartitionSpec
    from jax.experimental.shard_map import shard_map
    from concourse import bass2jax, mybir

    bass2jax.install_neuronx_cc_hook()

    partition_name = (nc.partition_id_tensor.name
                      if nc.partition_id_tensor else None)
    in_names, out_names, out_avals, zero_shapes = [], [], [], []
    for alloc in nc.m.functions[0].allocations:
        if not isinstance(alloc, mybir.MemoryLocationSet):
            continue
        name = alloc.memorylocations[0].name
        if alloc.kind == "ExternalInput":
            if name != partition_name:
                in_names.append(name)
        elif alloc.kind == "ExternalOutput":
            out_names.append(name)
            shape = tuple(alloc.tensor_shape)
            dt = mybir.dt.np(alloc.dtype)
            out_avals.append(jax.core.ShapedArray(shape, dt))
            zero_shapes.append((shape, dt))
    n_params = len(in_names)
    all_names = in_names + out_names
    if partition_name is not None:
        all_names = all_names + [partition_name]

    def _body(*args):
        operands = list(args)
        if partition_name is not None:
            operands.append(bass2jax.partition_id_tensor())
        outs = bass2jax._bass_exec_p.bind(
            *operands,
            out_avals=tuple(out_avals),
            in_names=tuple(all_names),
            out_names=tuple(out_names),
            lowering_input_output_aliases=(),
            sim_require_finite=True,
            sim_require_nnan=True,
            nc=nc,
        )
        return tuple(outs)

    devices = jax.devices()[:NCORES]
    mesh = Mesh(_np.asarray(devices), ("core",))
    n_outs = len(out_names)
    specs = (PartitionSpec("core"),) * (n_params + n_outs)
    donate = tuple(range(n_params, n_params + n_outs))
    sharded = jax.jit(
        shard_map(_body, mesh=mesh, in_specs=specs,
                  out_specs=(PartitionSpec("core"),) * n_outs,
                  check_rep=False),
        donate_argnums=donate, keep_unused=True)

    def run(feeds):
        """feeds: dict name -> full concatenated array [NCORES*dim0, ...].
        Returns dict name -> full concatenated output array."""
        concat_in = [feeds[name] for name in in_names]
        concat_zeros = [
            _np.zeros((NCORES * s[0],) + tuple(s[1:]), dt)
            for (s, dt) in zero_shapes
        ]
        out_arrs = sharded(*concat_in, *concat_zeros)
        return {name: _np.asarray(out_arrs[i])
                for i, name in enumerate(out_names)}

    # exposed for benchmarking (test.py)
    run._sharded = sharded
    run._in_names = in_names
    run._zero_shapes = zero_shapes
    return run


def _get_runner(C, mode, has_bias):
    key = (C, mode, has_bias)
    if key not in _CACHE:
        nc = _build(C, mode, has_bias)
        _CACHE[key] = (nc, _make_runner(nc))
    return _CACHE[key]


def _route(x, Wg, bg, k):
    """Replicates the reference selector in fp32: softmax gate, top-k
    (stable, ties to lower index like jax.lax.top_k), aux loss."""
    logits = x @ Wg + bg
    m = logits.max(-1, keepdims=True)
    p = np.exp(logits - m)
    gate = p / p.sum(-1, keepdims=True)
    idx = np.argsort(-gate, axis=-1, kind="stable")[:, :k]      # [B, k]
    vals = np.take_along_axis(gate, idx, axis=-1)               # [B, k]
    row_sum = gate.sum(-1)
    aux = (np.var(row_sum) / (np.mean(row_sum) ** 2 + np.float32(1e-10)))
    return idx, vals, np.float32(aux)


def _split_into(dst, view, spec, F):
    """Write `view` [NCORES, 128, F] (fp32, any strides) into dst
    [NCORES*128, nsplit*F] as hi (and lo residual for split modes)."""
    d = dst.reshape(NCORES, 128, -1)
    d[:, :, :F] = view                      # cast fp32 -> device dtype
    if spec["nsplit"] == 2:
        d[:, :, F:] = view - d[:, :, :F].astype(np.float32)


def _prepare(x, Wg, bg, We, be, k, mode):
    """Route + dispatch: returns (C, in_maps, dest, inv, aux)."""
    spec = _mode_spec(mode)
    idx, vals, aux = _route(x, Wg, bg, k)

    ef = idx.ravel()
    wf = vals.ravel()
    tf = np.repeat(np.arange(B), k)
    order = np.argsort(ef, kind="stable")
    counts = np.bincount(ef, minlength=E)
    C = int(max(128, -(-counts.max() // 128) * 128))
    nblk = C // 128

    starts = np.zeros(E, dtype=np.int64)
    starts[1:] = np.cumsum(counts)[:-1]
    dest = ef[order] * C + (np.arange(B * k) - starts[ef[order]])  # unique
    inv = np.argsort(order, kind="stable")

    Xg = np.zeros((E * C, NIN), dtype=np.float32)
    Xg[dest] = x[tf[order]]
    cwg = np.zeros(E * C, dtype=np.float32)
    cwg[dest] = wf[order]

    npdt = spec["npdt"]
    ns = spec["nsplit"]
    XF = EPC * KCH * C
    WF = EPC * KCH * NOUT

    # swizzle straight into the concat-ready device feeds
    # x: [core][p][i][kc][c],  w: [core][p][i][kc][n],  cw: [core][p][i][t]
    xd = np.empty((NCORES * 128, ns * XF), npdt)
    _split_into(xd, Xg.reshape(NCORES, EPC, C, KCH, 128)
                .transpose(0, 4, 1, 3, 2).reshape(NCORES, 128, XF), spec, XF)
    wd = np.empty((NCORES * 128, ns * WF), npdt)
    _split_into(wd, We.reshape(NCORES, EPC, KCH, 128, NOUT)
                .transpose(0, 3, 1, 2, 4).reshape(NCORES, 128, WF), spec, WF)
    csw = np.ascontiguousarray(
        cwg.reshape(NCORES, EPC, nblk, 128).transpose(0, 3, 1, 2)
        .reshape(NCORES * 128, EPC * nblk))

    has_bias = bool(np.any(be))
    feeds = {"cw": csw, "xd": xd, "wd": wd}
    if has_bias:
        feeds["be"] = np.ascontiguousarray(be.reshape(E, NOUT))
    return C, has_bias, feeds, dest, inv, aux


def kernel(x, Wg, bg, We, be, k):
    x = np.ascontiguousarray(np.asarray(x, dtype=np.float32))
    Wg = np.asarray(Wg, dtype=np.float32)
    bg = np.asarray(bg, dtype=np.float32)
    We = np.ascontiguousarray(np.asarray(We, dtype=np.float32))
    be = np.ascontiguousarray(np.asarray(be, dtype=np.float32))
    k = int(k)

    C, has_bias, feeds, dest, inv, aux = _prepare(x, Wg, bg, We, be, k, MODE)
    _, run = _get_runner(C, MODE, has_bias)
    results = run(feeds)

    Y = results["out"]                                  # [E*C, NOUT] global
    slot_rows = Y[dest[inv]]                            # token-major slots
    out = slot_rows.reshape(B, k, NOUT).sum(axis=1, dtype=np.float32)
    return out, aux


# revision 29
# speedup vs baseline: 1.7424x; 1.0849x over previous
"""MoE (top-2 of 32 experts, 512->512) on 8 NeuronCores, expert-parallel.

Strategy (full-I/O contract: kernel() receives full inputs, returns full output):
  - Host computes the small selector (softmax gate + top-k) in fp32 numpy,
    mirroring the reference op-for-op, and performs the "all-to-all dispatch":
    tokens are grouped by expert into capacity-padded batches (the host-side
    sharding step of the expert-parallel layout).
  - Experts are sharded 4-per-core across the 8 cores.  Each core runs a Bass
    kernel: for each of its experts, Y = relu(X_e @ We[e] + be[e]) * gate_w,
    with the per-token gate weight fused into the relu via the ScalarE
    activation (scale is per-partition; gate weights are >= 0 so
    w*relu(z) == relu(w*z)).
  - Host combines: out[token] = sum of its k slot rows (weighted on device).

Precision modes for the expert GEMMs (MODE):
  - "f32"  : exact fp32 matmuls (PE runs them as 2 half-rate passes).
  - "f16x3": x and W split into fp16 hi+lo; y = xh@wh + xh@wl + xl@wh.
             fp16 products are exact in fp32 (11+11 mantissa bits < 24), so
             the only extra error is the dropped xl@wl term (~2^-22 relative)
             - fp32-equivalent accuracy at 3/4 the PE cost of "f32".
  - "f32r" : single-pass relaxed-precision fp32 (tf32-like, ~1e-4 rel err).
  - "bf16" : single-pass bf16 (~1e-3 rel err).

All tensors are pre-swizzled on the host into the exact SBUF layout so every
device DMA is a contiguous copy.  The device kernel is compiled once per
(capacity, mode, has_bias) and cached.
"""

import numpy as np
from contextlib import ExitStack

B, NIN, NOUT, E, NCORES = 8192, 512, 512, 32, 8
EPC = E // NCORES  # experts per core
KCH = NIN // 128   # contraction chunks of 128

MODE = "f16x3"

_CACHE = {}


def _mode_spec(mode):
    import concourse.mybir as mybir
    import ml_dtypes
    if mode == "f32":
        return dict(dt=mybir.dt.float32, npdt=np.float32, nsplit=1,
                    passes=[(0, 0)])
    if mode == "f32r":
        return dict(dt=mybir.dt.float32r, npdt=np.float32, nsplit=1,
                    passes=[(0, 0)])
    if mode == "bf16":
        return dict(dt=mybir.dt.bfloat16, npdt=ml_dtypes.bfloat16, nsplit=1,
                    passes=[(0, 0)])
    if mode == "f16x3":
        return dict(dt=mybir.dt.float16, npdt=np.float16, nsplit=2,
                    passes=[(0, 0), (0, 1), (1, 0)])
    raise ValueError(mode)


def _build(caps, mode, has_bias):
    """Build + compile the per-core Bass program.  caps[j] = number of
    128-token blocks for expert slot j (same structure on every core; the
    host assigns its busiest expert to slot 0 etc. so capacity is not
    wasted padding every expert to the global max)."""
    import concourse.mybir as mybir
    import concourse.tile as tile
    from concourse import bacc

    spec = _mode_spec(mode)
    mmdt = spec["dt"]
    nsplit = spec["nsplit"]
    passes = spec["passes"]
    Cj = [c * 128 for c in caps]           # tokens per slot
    CT = sum(Cj)                           # tokens per core
    NBLK = sum(caps)
    ROFF = np.concatenate([[0], np.cumsum(Cj)]).astype(int)   # token offsets
    BOFF = np.concatenate([[0], np.cumsum(caps)]).astype(int)  # block offsets
    XOFF = [KCH * r for r in ROFF]         # x free-elem offsets per slot
    XF = KCH * CT           # x free elems per split: [p][slot][kc][c]
    WF = EPC * KCH * NOUT   # w free elems per split: [p][slot][kc][n]

    nc = bacc.Bacc("TRN2", target_bir_lowering=False, debug=False,
                   num_devices=NCORES)
    # hi/lo splits are stacked along the free axis of ONE tensor so a single
    # DMA moves both (fewer DMA triggers - they serialize on the HWDGE ring)
    xd = nc.dram_tensor("xd", [128, nsplit * XF], mmdt,
                        kind="ExternalInput").ap()
    wd = nc.dram_tensor("wd", [128, nsplit * WF], mmdt,
                        kind="ExternalInput").ap()
    cw = nc.dram_tensor("cw", [128, NBLK], mybir.dt.float32,
                        kind="ExternalInput").ap()
    if has_bias:
        be = nc.dram_tensor("be", [EPC, NOUT], mybir.dt.float32,
                            kind="ExternalInput").ap()
    out = nc.dram_tensor("out", [CT, NOUT], mybir.dt.float32,
                         kind="ExternalOutput").ap()

    with tile.TileContext(nc) as tc, ExitStack() as ctx:
        xpool = ctx.enter_context(tc.tile_pool(name="x", bufs=1))
        wpool = ctx.enter_context(tc.tile_pool(name="w", bufs=1))
        spool = ctx.enter_context(tc.tile_pool(name="s", bufs=1))
        opool = ctx.enter_context(tc.tile_pool(name="o", bufs=1))
        pspool = ctx.enter_context(tc.tile_pool(name="ps", bufs=1,
                                                space="PSUM"))

        x_all = xpool.tile([128, nsplit * XF], mmdt, name="x_all")
        w_all = wpool.tile([128, nsplit * WF], mmdt, name="w_all")
        x_sb = [x_all[:, s * XF:(s + 1) * XF] for s in range(nsplit)]
        w_sb = [w_all[:, s * WF:(s + 1) * WF] for s in range(nsplit)]

        # cw (and bias) first: tiny, and every ACT epilogue needs it - behind
        # the bulk inputs on the FIFO ring it would stall all PSUM recycling.
        cw_sb = spool.tile([128, NBLK], mybir.dt.float32)
        nc.sync.dma_start(cw_sb[:], cw)
        if has_bias:
            bias_sb = spool.tile([EPC, NOUT], mybir.dt.float32)
            nc.sync.dma_start(bias_sb[:], be)
            ones_sb = spool.tile([1, 128], mybir.dt.float32)
            nc.vector.memset(ones_sb[:], 1.0)

        # Input DMAs per (expert, kc-half): one DMA covers hi+lo (2 strided
        # regions).  Inputs ride the SP HWDGE ring; outputs ride the ACT ring
        # to avoid head-of-line blocking between the two streams.
        xv = xd.rearrange("p (s f) -> p s f", s=nsplit)
        wv = wd.rearrange("p (s f) -> p s f", s=nsplit)
        xav = x_all[:].rearrange("p (s f) -> p s f", s=nsplit)
        wav = w_all[:].rearrange("p (s f) -> p s f", s=nsplit)
        for j in range(EPC):
            if caps[j] == 0:
                continue
            # finer chunks for slot 0 so the first matmul starts sooner
            KH = 1 if j == 0 else 2
            for kh in range(0, KCH, KH):
                xl_ = XOFF[j] + kh * Cj[j]
                xr_ = XOFF[j] + (kh + KH) * Cj[j]
                wl_ = (j * KCH + kh) * NOUT
                wr_ = (j * KCH + kh + KH) * NOUT
                nc.sync.dma_start(xav[:, :, xl_:xr_], xv[:, :, xl_:xr_])
                nc.sync.dma_start(wav[:, :, wl_:wr_], wv[:, :, wl_:wr_])

        out_sb = opool.tile([128, NBLK * NOUT], mybir.dt.float32)

        npass = len(passes)
        nmm = KCH * npass + (1 if has_bias else 0)
        for j in range(EPC):
            nblk = caps[j]
            if nblk == 0:
                continue
            # k-chunk outer, token-block inner: each block's PSUM bank stays
            # live across the kc sweep, so the first matmuls only need the
            # first (x, w) k-chunk of this slot.
            pss = [pspool.tile([128, NOUT], mybir.dt.float32,
                               name=f"ps_{j}_{t}", tag=f"ps{t}")
                   for t in range(nblk)]
            for kc in range(KCH):
                xoff = XOFF[j] + kc * Cj[j]
                woff = (j * KCH + kc) * NOUT
                for t in range(nblk):
                    for p, (sx, sw) in enumerate(passes):
                        m = kc * npass + p + 1
                        nc.tensor.matmul(
                            pss[t][:],
                            lhsT=x_sb[sx][:, xoff + t * 128:
                                          xoff + (t + 1) * 128],
                            rhs=w_sb[sw][:, woff:woff + NOUT],
                            start=(m == 1), stop=(m == nmm))
            for t in range(nblk):
                blk = BOFF[j] + t
                if has_bias:
                    nc.tensor.matmul(
                        pss[t][:], lhsT=ones_sb[:], rhs=bias_sb[j:j + 1, :],
                        start=False, stop=True)
                nc.scalar.activation(
                    out_sb[:, blk * NOUT:(blk + 1) * NOUT], pss[t][:],
                    mybir.ActivationFunctionType.Relu,
                    scale=cw_sb[:, blk:blk + 1])
            # out DMAs per slot-half (fewer triggers); per-block for the
            # last slot so the kernel tail is short
            if j < EPC - 1:
                tranges = ((0, nblk - nblk // 2), (nblk - nblk // 2, nblk))
            else:
                tranges = tuple((t, t + 1) for t in range(nblk))
            for (t0, t1) in tranges:
                nc.scalar.dma_start(
                    out[ROFF[j] + t0 * 128:ROFF[j] + t1 * 128, :]
                    .rearrange("(t p) n -> p t n", p=128),
                    out_sb[:].rearrange("p (b n) -> p b n", n=NOUT)
                    [:, BOFF[j] + t0:BOFF[j] + t1, :])

    nc.compile()
    return nc


def _make_runner(nc):
    """One-time jit of the 8-core SPMD executable (mirrors
    bass2jax.run_bass_via_pjrt, cached so repeat calls skip retracing)."""
    import jax
    import jax.core
    import numpy as _np
    from jax.sharding import Mesh, PartitionSpec
    from jax.experimental.shard_map import shard_map
    from concourse import bass2jax, mybir

    bass2jax.install_neuronx_cc_hook()

    partition_name = (nc.partition_id_tensor.name
                      if nc.partition_id_tensor else None)
    in_names, out_names, out_avals, zero_shapes = [], [], [], []
    for alloc in nc.m.functions[0].allocations:
        if not isinstance(alloc, mybir.MemoryLocationSet):
            continue
        name = alloc.memorylocations[0].name
        if alloc.kind == "ExternalInput":
            if name != partition_name:
                in_names.append(name)
        elif alloc.kind == "ExternalOutput":
            out_names.append(name)
            shape = tuple(alloc.tensor_shape)
            dt = mybir.dt.np(alloc.dtype)
            out_avals.append(jax.core.ShapedArray(shape, dt))
            zero_shapes.append((shape, dt))
    n_params = len(in_names)
    all_names = in_names + out_names
    if partition_name is not None:
        all_names = all_names + [partition_name]

    def _body(*args):
        operands = list(args)
        if partition_name is not None:
            operands.append(bass2jax.partition_id_tensor())
        outs = bass2jax._bass_exec_p.bind(
            *operands,
            out_avals=tuple(out_avals),
            in_names=tuple(all_names),
            out_names=tuple(out_names),
            lowering_input_output_aliases=(),
            sim_require_finite=True,
            sim_require_nnan=True,
            nc=nc,
        )
        return tuple(outs)

    devices = jax.devices()[:NCORES]
    mesh = Mesh(_np.asarray(devices), ("core",))
    n_outs = len(out_names)
    specs = (PartitionSpec("core"),) * (n_params + n_outs)
    donate = tuple(range(n_params, n_params + n_outs))
    sharded = jax.jit(
        shard_map(_body, mesh=mesh, in_specs=specs,
                  out_specs=(PartitionSpec("core"),) * n_outs,
                  check_rep=False),
        donate_argnums=donate, keep_unused=True)

    def run(feeds):
        """feeds: dict name -> full concatenated array [NCORES*dim0, ...].
        Returns dict name -> full concatenated output array."""
        concat_in = [feeds[name] for name in in_names]
        concat_zeros = [
            _np.zeros((NCORES * s[0],) + tuple(s[1:]), dt)
            for (s, dt) in zero_shapes
        ]
        out_arrs = sharded(*concat_in, *concat_zeros)
        return {name: _np.asarray(out_arrs[i])
                for i, name in enumerate(out_names)}

    # exposed for benchmarking (test.py)
    run._sharded = sharded
    run._in_names = in_names
    run._zero_shapes = zero_shapes
    return run


def _get_runner(caps, mode, has_bias):
    key = (caps, mode, has_bias)
    if key not in _CACHE:
        nc = _build(caps, mode, has_bias)
        _CACHE[key] = (nc, _make_runner(nc))
    return _CACHE[key]


def _route(x, Wg, bg, k):
    """Replicates the reference selector in fp32: softmax gate, top-k
    (stable, ties to lower index like jax.lax.top_k), aux loss."""
    logits = x @ Wg + bg
    m = logits.max(-1, keepdims=True)
    p = np.exp(logits - m)
    gate = p / p.sum(-1, keepdims=True)
    idx = np.argsort(-gate, axis=-1, kind="stable")[:, :k]      # [B, k]
    vals = np.take_along_axis(gate, idx, axis=-1)               # [B, k]
    row_sum = gate.sum(-1)
    aux = (np.var(row_sum) / (np.mean(row_sum) ** 2 + np.float32(1e-10)))
    return idx, vals, np.float32(aux)


def _split_into(dst, view, spec, F):
    """Write `view` [NCORES, 128, F] (fp32, any strides) into dst
    [NCORES*128, nsplit*F] as hi (and lo residual for split modes)."""
    d = dst.reshape(NCORES, 128, -1)
    d[:, :, :F] = view                      # cast fp32 -> device dtype
    if spec["nsplit"] == 2:
        d[:, :, F:] = view - d[:, :, :F].astype(np.float32)


def _prepare(x, Wg, bg, We, be, k, mode):
    """Route + dispatch: returns (caps, has_bias, feeds, dest, inv, aux)."""
    spec = _mode_spec(mode)
    idx, vals, aux = _route(x, Wg, bg, k)

    ef = idx.ravel()
    wf = vals.ravel()
    tf = np.repeat(np.arange(B), k)
    order = np.argsort(ef, kind="stable")
    counts = np.bincount(ef, minlength=E)

    # Load-sorted slot assignment: expert with load-rank r goes to core r%8,
    # slot r//8; slot j's capacity is the max block count in rank octile j,
    # so every core compiles to the same (caps) block structure.
    blocks = np.maximum(1, -(-counts // 128))
    rank_order = np.argsort(-counts, kind="stable")      # expert ids by load
    caps = tuple(int(blocks[rank_order[NCORES * j]]) for j in range(EPC))
    Cj = np.array([c * 128 for c in caps])
    CT = int(Cj.sum())
    NBLK = sum(caps)
    ROFF = np.concatenate([[0], np.cumsum(Cj)]).astype(np.int64)

    core_of = np.empty(E, dtype=np.int64)
    slot_of = np.empty(E, dtype=np.int64)
    core_of[rank_order] = np.arange(E) % NCORES
    slot_of[rank_order] = np.arange(E) // NCORES

    starts = np.zeros(E, dtype=np.int64)
    starts[1:] = np.cumsum(counts)[:-1]
    base = core_of * CT + ROFF[slot_of]                  # per-expert row base
    es = ef[order]
    dest = base[es] + (np.arange(B * k) - starts[es])    # unique global rows
    inv = np.argsort(order, kind="stable")

    Xg = np.zeros((NCORES * CT, NIN), dtype=np.float32)
    Xg[dest] = x[tf[order]]
    cwg = np.zeros(NCORES * CT, dtype=np.float32)
    cwg[dest] = wf[order]

    npdt = spec["npdt"]
    ns = spec["nsplit"]
    XF = KCH * CT
    WF = EPC * KCH * NOUT

    # swizzle straight into the concat-ready device feeds
    # x: [core][p][slot][kc][c] with per-slot capacities
    xsw = np.empty((NCORES, 128, XF), np.float32)
    Xg3 = Xg.reshape(NCORES, CT, NIN)
    for j in range(EPC):
        xsw[:, :, KCH * ROFF[j]:KCH * ROFF[j + 1]] = (
            Xg3[:, ROFF[j]:ROFF[j + 1], :]
            .reshape(NCORES, Cj[j], KCH, 128).transpose(0, 3, 2, 1)
            .reshape(NCORES, 128, KCH * Cj[j]))
    xd = np.empty((NCORES * 128, ns * XF), npdt)
    _split_into(xd, xsw, spec, XF)

    # w: [core][p][slot][kc][n]; core c's slot j holds expert rank_order[8j+c]
    eid = rank_order.reshape(EPC, NCORES).T              # [core, slot]
    wd = np.empty((NCORES * 128, ns * WF), npdt)
    _split_into(wd, We[eid].reshape(NCORES, EPC, KCH, 128, NOUT)
                .transpose(0, 3, 1, 2, 4).reshape(NCORES, 128, WF), spec, WF)

    # cw: [core][p][global block]
    csw = np.empty((NCORES, 128, NBLK), np.float32)
    cw3 = cwg.reshape(NCORES, CT)
    boff = 0
    for j in range(EPC):
        csw[:, :, boff:boff + caps[j]] = (
            cw3[:, ROFF[j]:ROFF[j + 1]]
            .reshape(NCORES, caps[j], 128).transpose(0, 2, 1))
        boff += caps[j]

    has_bias = bool(np.any(be))
    feeds = {"cw": csw.reshape(NCORES * 128, NBLK), "xd": xd, "wd": wd}
    if has_bias:
        feeds["be"] = np.ascontiguousarray(be[eid.ravel()])
    return caps, has_bias, feeds, dest, inv, aux


def kernel(x, Wg, bg, We, be, k):
    x = np.ascontiguousarray(np.asarray(x, dtype=np.float32))
    Wg = np.asarray(Wg, dtype=np.float32)
    bg = np.asarray(bg, dtype=np.float32)
    We = np.ascontiguousarray(np.asarray(We, dtype=np.float32))
    be = np.ascontiguousarray(np.asarray(be, dtype=np.float32))
    k = int(k)

    caps, has_bias, feeds, dest, inv, aux = _prepare(x, Wg, bg, We, be, k,
                                                     MODE)
    _, run = _get_runner(caps, MODE, has_bias)
    results = run(feeds)

    Y = results["out"]                                  # [E*C, NOUT] global
    slot_rows = Y[dest[inv]]                            # token-major slots
    out = slot_rows.reshape(B, k, NOUT).sum(axis=1, dtype=np.float32)
    return out, aux


# revision 38
# speedup vs baseline: 1.8716x; 1.0742x over previous
"""MoE (top-2 of 32 experts, 512->512) on 8 NeuronCores, expert-parallel.

Strategy (full-I/O contract: kernel() receives full inputs, returns full output):
  - Host computes the small selector (softmax gate + top-k) in fp32 numpy,
    mirroring the reference op-for-op, and performs the "all-to-all dispatch":
    tokens are grouped by expert into capacity-padded batches (the host-side
    sharding step of the expert-parallel layout).
  - Experts are sharded 4-per-core across the 8 cores.  Each core runs a Bass
    kernel: for each of its experts, Y = relu(X_e @ We[e] + be[e]) * gate_w,
    with the per-token gate weight fused into the relu via the ScalarE
    activation (scale is per-partition; gate weights are >= 0 so
    w*relu(z) == relu(w*z)).
  - Host combines: out[token] = sum of its k slot rows (weighted on device).

Precision modes for the expert GEMMs (MODE):
  - "f32"  : exact fp32 matmuls (PE runs them as 2 half-rate passes).
  - "f16x3": x and W split into fp16 hi+lo; y = xh@wh + xh@wl + xl@wh.
             fp16 products are exact in fp32 (11+11 mantissa bits < 24), so
             the only extra error is the dropped xl@wl term (~2^-22 relative)
             - fp32-equivalent accuracy at 3/4 the PE cost of "f32".
  - "f32r" : single-pass relaxed-precision fp32 (tf32-like, ~1e-4 rel err).
  - "bf16" : single-pass bf16 (~1e-3 rel err).

All tensors are pre-swizzled on the host into the exact SBUF layout so every
device DMA is a contiguous copy.  The device kernel is compiled once per
(capacity, mode, has_bias) and cached.
"""

import numpy as np
from contextlib import ExitStack

B, NIN, NOUT, E, NCORES = 8192, 512, 512, 32, 8
EPC = E // NCORES  # experts per core
KCH = NIN // 128   # contraction chunks of 128

MODE = "f16x3"

_CACHE = {}


def _mode_spec(mode):
    import concourse.mybir as mybir
    import ml_dtypes
    if mode == "f32":
        return dict(dt=mybir.dt.float32, npdt=np.float32, nsplit=1,
                    passes=[(0, 0)])
    if mode == "f32r":
        return dict(dt=mybir.dt.float32r, npdt=np.float32, nsplit=1,
                    passes=[(0, 0)])
    if mode == "bf16":
        return dict(dt=mybir.dt.bfloat16, npdt=ml_dtypes.bfloat16, nsplit=1,
                    passes=[(0, 0)])
    if mode == "f16x3":
        return dict(dt=mybir.dt.float16, npdt=np.float16, nsplit=2,
                    passes=[(0, 0), (0, 1), (1, 0)])
    raise ValueError(mode)


def _build(caps, mode, has_bias):
    """Build + compile the per-core Bass program.  caps[j] = number of
    128-token blocks for expert slot j (same structure on every core; the
    host assigns its busiest expert to slot 0 etc. so capacity is not
    wasted padding every expert to the global max)."""
    import concourse.mybir as mybir
    import concourse.tile as tile
    from concourse import bacc

    spec = _mode_spec(mode)
    mmdt = spec["dt"]
    nsplit = spec["nsplit"]
    passes = spec["passes"]
    Cj = [c * 128 for c in caps]           # tokens per slot
    CT = sum(Cj)                           # tokens per core
    NBLK = sum(caps)
    ROFF = np.concatenate([[0], np.cumsum(Cj)]).astype(int)   # token offsets
    BOFF = np.concatenate([[0], np.cumsum(caps)]).astype(int)  # block offsets
    XOFF = [KCH * r for r in ROFF]         # x free-elem offsets per slot
    XF = KCH * CT           # x free elems per split: [p][slot][kc][c]
    WF = EPC * KCH * NOUT   # w free elems per split: [p][slot][kc][n]

    nc = bacc.Bacc("TRN2", target_bir_lowering=False, debug=False,
                   num_devices=NCORES)
    # hi/lo splits are stacked along the free axis of ONE tensor so a single
    # DMA moves both (fewer DMA triggers - they serialize on the HWDGE ring)
    xd = nc.dram_tensor("xd", [128, nsplit * XF], mmdt,
                        kind="ExternalInput").ap()
    wd = nc.dram_tensor("wd", [128, nsplit * WF], mmdt,
                        kind="ExternalInput").ap()
    cw = nc.dram_tensor("cw", [128, NBLK], mybir.dt.float32,
                        kind="ExternalInput").ap()
    if has_bias:
        be = nc.dram_tensor("be", [1, EPC * NOUT], mybir.dt.float32,
                            kind="ExternalInput").ap()
    out = nc.dram_tensor("out", [CT, NOUT], mybir.dt.float32,
                         kind="ExternalOutput").ap()

    with tile.TileContext(nc) as tc, ExitStack() as ctx:
        xpool = ctx.enter_context(tc.tile_pool(name="x", bufs=1))
        wpool = ctx.enter_context(tc.tile_pool(name="w", bufs=1))
        spool = ctx.enter_context(tc.tile_pool(name="s", bufs=1))
        opool = ctx.enter_context(tc.tile_pool(name="o", bufs=1))
        pspool = ctx.enter_context(tc.tile_pool(name="ps", bufs=1,
                                                space="PSUM"))

        x_all = xpool.tile([128, nsplit * XF], mmdt, name="x_all")
        w_all = wpool.tile([128, nsplit * WF], mmdt, name="w_all")
        x_sb = [x_all[:, s * XF:(s + 1) * XF] for s in range(nsplit)]
        w_sb = [w_all[:, s * WF:(s + 1) * WF] for s in range(nsplit)]

        # cw (and bias) first: tiny, and every ACT epilogue needs it - behind
        # the bulk inputs on the FIFO ring it would stall all PSUM recycling.
        cw_sb = spool.tile([128, NBLK], mybir.dt.float32)
        nc.sync.dma_start(cw_sb[:], cw)
        if has_bias:
            bias_sb = spool.tile([1, EPC * NOUT], mybir.dt.float32)
            nc.sync.dma_start(bias_sb[:], be)
            ones_sb = spool.tile([1, 128], mybir.dt.float32)
            nc.vector.memset(ones_sb[:], 1.0)

        # Input DMAs per (expert, kc-half): one DMA covers hi+lo (2 strided
        # regions).  Inputs ride the SP HWDGE ring; outputs ride the ACT ring
        # to avoid head-of-line blocking between the two streams.
        xv = xd.rearrange("p (s f) -> p s f", s=nsplit)
        wv = wd.rearrange("p (s f) -> p s f", s=nsplit)
        xav = x_all[:].rearrange("p (s f) -> p s f", s=nsplit)
        wav = w_all[:].rearrange("p (s f) -> p s f", s=nsplit)
        for j in range(EPC):
            if caps[j] == 0:
                continue
            # finer chunks for slot 0 so the first matmul starts sooner
            KH = 1 if j == 0 else 2
            for kh in range(0, KCH, KH):
                xl_ = XOFF[j] + kh * Cj[j]
                xr_ = XOFF[j] + (kh + KH) * Cj[j]
                wl_ = (j * KCH + kh) * NOUT
                wr_ = (j * KCH + kh + KH) * NOUT
                nc.sync.dma_start(xav[:, :, xl_:xr_], xv[:, :, xl_:xr_])
                nc.sync.dma_start(wav[:, :, wl_:wr_], wv[:, :, wl_:wr_])

        out_sb = opool.tile([128, NBLK * NOUT], mybir.dt.float32)

        # PE warmup: dependency-free dummy matmuls ramp the PE clock to full
        # speed while the first input DMAs are still in flight.
        wu_sb = spool.tile([128, NOUT], mmdt)
        nc.vector.memset(wu_sb[:], 0.0)
        wu_ps = pspool.tile([128, NOUT], mybir.dt.float32, tag="ps_wu")
        for _ in range(11):
            nc.tensor.matmul(wu_ps[:], lhsT=wu_sb[:, :128], rhs=wu_sb[:],
                             start=True, stop=True)

        npass = len(passes)
        nmm = KCH * npass + (1 if has_bias else 0)
        for j in range(EPC):
            nblk = caps[j]
            if nblk == 0:
                continue
            # Slot 0 runs k-chunk outer (each block's PSUM bank stays live
            # across the kc sweep) so the first matmuls only need the first
            # (x, w) k-chunk.  Later slots have all data resident by the
            # time they run, so they go block-major, which spreads the ACT +
            # out-DMA epilogues instead of bunching them at the slot end.
            pss = [pspool.tile([128, NOUT], mybir.dt.float32,
                               name=f"ps_{j}_{t}", tag=f"ps{t}")
                   for t in range(nblk)]
            if j == 0:
                mm_order = [(kc, t) for kc in range(KCH)
                            for t in range(nblk)]
            else:
                mm_order = [(kc, t) for t in range(nblk)
                            for kc in range(KCH)]
            for (kc, t) in mm_order:
                xoff = XOFF[j] + kc * Cj[j]
                woff = (j * KCH + kc) * NOUT
                for p, (sx, sw) in enumerate(passes):
                    m = kc * npass + p + 1
                    nc.tensor.matmul(
                        pss[t][:],
                        lhsT=x_sb[sx][:, xoff + t * 128:
                                      xoff + (t + 1) * 128],
                        rhs=w_sb[sw][:, woff:woff + NOUT],
                        start=(m == 1), stop=(m == nmm))
            for t in range(nblk):
                blk = BOFF[j] + t
                if has_bias:
                    nc.tensor.matmul(
                        pss[t][:], lhsT=ones_sb[:],
                        rhs=bias_sb[:, j * NOUT:(j + 1) * NOUT],
                        start=False, stop=True)
                nc.scalar.activation(
                    out_sb[:, blk * NOUT:(blk + 1) * NOUT], pss[t][:],
                    mybir.ActivationFunctionType.Relu,
                    scale=cw_sb[:, blk:blk + 1])
            # out DMAs per slot-half (fewer triggers); per-block for the
            # last slot so the kernel tail is short
            if j < EPC - 1:
                tranges = ((0, nblk - nblk // 2), (nblk - nblk // 2, nblk))
            else:
                tranges = tuple((t, t + 1) for t in range(nblk))
            for (t0, t1) in tranges:
                nc.scalar.dma_start(
                    out[ROFF[j] + t0 * 128:ROFF[j] + t1 * 128, :]
                    .rearrange("(t p) n -> p t n", p=128),
                    out_sb[:].rearrange("p (b n) -> p b n", n=NOUT)
                    [:, BOFF[j] + t0:BOFF[j] + t1, :])

    nc.compile()
    return nc


def _make_runner(nc):
    """One-time jit of the 8-core SPMD executable (mirrors
    bass2jax.run_bass_via_pjrt, cached so repeat calls skip retracing)."""
    import jax
    import jax.core
    import numpy as _np
    from jax.sharding import Mesh, PartitionSpec
    from jax.experimental.shard_map import shard_map
    from concourse import bass2jax, mybir

    bass2jax.install_neuronx_cc_hook()

    partition_name = (nc.partition_id_tensor.name
                      if nc.partition_id_tensor else None)
    in_names, out_names, out_avals, zero_shapes = [], [], [], []
    for alloc in nc.m.functions[0].allocations:
        if not isinstance(alloc, mybir.MemoryLocationSet):
            continue
        name = alloc.memorylocations[0].name
        if alloc.kind == "ExternalInput":
            if name != partition_name:
                in_names.append(name)
        elif alloc.kind == "ExternalOutput":
            out_names.append(name)
            shape = tuple(alloc.tensor_shape)
            dt = mybir.dt.np(alloc.dtype)
            out_avals.append(jax.core.ShapedArray(shape, dt))
            zero_shapes.append((shape, dt))
    n_params = len(in_names)
    all_names = in_names + out_names
    if partition_name is not None:
        all_names = all_names + [partition_name]

    def _body(*args):
        operands = list(args)
        if partition_name is not None:
            operands.append(bass2jax.partition_id_tensor())
        outs = bass2jax._bass_exec_p.bind(
            *operands,
            out_avals=tuple(out_avals),
            in_names=tuple(all_names),
            out_names=tuple(out_names),
            lowering_input_output_aliases=(),
            sim_require_finite=True,
            sim_require_nnan=True,
            nc=nc,
        )
        return tuple(outs)

    devices = jax.devices()[:NCORES]
    mesh = Mesh(_np.asarray(devices), ("core",))
    n_outs = len(out_names)
    specs = (PartitionSpec("core"),) * (n_params + n_outs)
    donate = tuple(range(n_params, n_params + n_outs))
    sharded = jax.jit(
        shard_map(_body, mesh=mesh, in_specs=specs,
                  out_specs=(PartitionSpec("core"),) * n_outs,
                  check_rep=False),
        donate_argnums=donate, keep_unused=True)

    def run(feeds):
        """feeds: dict name -> full concatenated array [NCORES*dim0, ...].
        Returns dict name -> full concatenated output array."""
        concat_in = [feeds[name] for name in in_names]
        concat_zeros = [
            _np.zeros((NCORES * s[0],) + tuple(s[1:]), dt)
            for (s, dt) in zero_shapes
        ]
        out_arrs = sharded(*concat_in, *concat_zeros)
        return {name: _np.asarray(out_arrs[i])
                for i, name in enumerate(out_names)}

    # exposed for benchmarking (test.py)
    run._sharded = sharded
    run._in_names = in_names
    run._zero_shapes = zero_shapes
    return run


def _get_runner(caps, mode, has_bias):
    key = (caps, mode, has_bias)
    if key not in _CACHE:
        nc = _build(caps, mode, has_bias)
        _CACHE[key] = (nc, _make_runner(nc))
    return _CACHE[key]


def _route(x, Wg, bg, k):
    """Replicates the reference selector in fp32: softmax gate, top-k
    (stable, ties to lower index like jax.lax.top_k), aux loss."""
    logits = x @ Wg + bg
    m = logits.max(-1, keepdims=True)
    p = np.exp(logits - m)
    gate = p / p.sum(-1, keepdims=True)
    idx = np.argsort(-gate, axis=-1, kind="stable")[:, :k]      # [B, k]
    vals = np.take_along_axis(gate, idx, axis=-1)               # [B, k]
    row_sum = gate.sum(-1)
    aux = (np.var(row_sum) / (np.mean(row_sum) ** 2 + np.float32(1e-10)))
    return idx, vals, np.float32(aux)


def _split_into(dst, view, spec, F):
    """Write `view` [NCORES, 128, F] (fp32, any strides) into dst
    [NCORES*128, nsplit*F] as hi (and lo residual for split modes)."""
    d = dst.reshape(NCORES, 128, -1)
    d[:, :, :F] = view                      # cast fp32 -> device dtype
    if spec["nsplit"] == 2:
        d[:, :, F:] = view - d[:, :, :F].astype(np.float32)


def _prepare(x, Wg, bg, We, be, k, mode):
    """Route + dispatch: returns (caps, has_bias, feeds, dest, inv, aux)."""
    spec = _mode_spec(mode)
    idx, vals, aux = _route(x, Wg, bg, k)

    ef = idx.ravel()
    wf = vals.ravel()
    tf = np.repeat(np.arange(B), k)
    order = np.argsort(ef, kind="stable")
    counts = np.bincount(ef, minlength=E)

    # Load-sorted slot assignment: expert with load-rank r goes to core r%8,
    # slot r//8; slot j's capacity is the max block count in rank octile j,
    # so every core compiles to the same (caps) block structure.
    blocks = np.maximum(1, -(-counts // 128))
    rank_order = np.argsort(-counts, kind="stable")      # expert ids by load
    caps = tuple(int(blocks[rank_order[NCORES * j]]) for j in range(EPC))
    Cj = np.array([c * 128 for c in caps])
    CT = int(Cj.sum())
    NBLK = sum(caps)
    ROFF = np.concatenate([[0], np.cumsum(Cj)]).astype(np.int64)

    core_of = np.empty(E, dtype=np.int64)
    slot_of = np.empty(E, dtype=np.int64)
    core_of[rank_order] = np.arange(E) % NCORES
    slot_of[rank_order] = np.arange(E) // NCORES

    starts = np.zeros(E, dtype=np.int64)
    starts[1:] = np.cumsum(counts)[:-1]
    base = core_of * CT + ROFF[slot_of]                  # per-expert row base
    es = ef[order]
    dest = base[es] + (np.arange(B * k) - starts[es])    # unique global rows
    inv = np.argsort(order, kind="stable")

    Xg = np.zeros((NCORES * CT, NIN), dtype=np.float32)
    Xg[dest] = x[tf[order]]
    cwg = np.zeros(NCORES * CT, dtype=np.float32)
    cwg[dest] = wf[order]

    npdt = spec["npdt"]
    ns = spec["nsplit"]
    XF = KCH * CT
    WF = EPC * KCH * NOUT

    # swizzle straight into the concat-ready device feeds
    # x: [core][p][slot][kc][c] with per-slot capacities
    xsw = np.empty((NCORES, 128, XF), np.float32)
    Xg3 = Xg.reshape(NCORES, CT, NIN)
    for j in range(EPC):
        xsw[:, :, KCH * ROFF[j]:KCH * ROFF[j + 1]] = (
            Xg3[:, ROFF[j]:ROFF[j + 1], :]
            .reshape(NCORES, Cj[j], KCH, 128).transpose(0, 3, 2, 1)
            .reshape(NCORES, 128, KCH * Cj[j]))
    xd = np.empty((NCORES * 128, ns * XF), npdt)
    _split_into(xd, xsw, spec, XF)

    # w: [core][p][slot][kc][n]; core c's slot j holds expert rank_order[8j+c]
    eid = rank_order.reshape(EPC, NCORES).T              # [core, slot]
    wd = np.empty((NCORES * 128, ns * WF), npdt)
    _split_into(wd, We[eid].reshape(NCORES, EPC, KCH, 128, NOUT)
                .transpose(0, 3, 1, 2, 4).reshape(NCORES, 128, WF), spec, WF)

    # cw: [core][p][global block]
    csw = np.empty((NCORES, 128, NBLK), np.float32)
    cw3 = cwg.reshape(NCORES, CT)
    boff = 0
    for j in range(EPC):
        csw[:, :, boff:boff + caps[j]] = (
            cw3[:, ROFF[j]:ROFF[j + 1]]
            .reshape(NCORES, caps[j], 128).transpose(0, 2, 1))
        boff += caps[j]

    has_bias = bool(np.any(be))
    feeds = {"cw": csw.reshape(NCORES * 128, NBLK), "xd": xd, "wd": wd}
    if has_bias:
        feeds["be"] = np.ascontiguousarray(
            be[eid].reshape(NCORES, EPC * NOUT))   # [core][1, e*n] rows
    return caps, has_bias, feeds, dest, inv, aux


def kernel(x, Wg, bg, We, be, k):
    x = np.ascontiguousarray(np.asarray(x, dtype=np.float32))
    Wg = np.asarray(Wg, dtype=np.float32)
    bg = np.asarray(bg, dtype=np.float32)
    We = np.ascontiguousarray(np.asarray(We, dtype=np.float32))
    be = np.ascontiguousarray(np.asarray(be, dtype=np.float32))
    k = int(k)

    caps, has_bias, feeds, dest, inv, aux = _prepare(x, Wg, bg, We, be, k,
                                                     MODE)
    _, run = _get_runner(caps, MODE, has_bias)
    results = run(feeds)

    Y = results["out"]                                  # [E*C, NOUT] global
    slot_rows = Y[dest[inv]]                            # token-major slots
    out = slot_rows.reshape(B, k, NOUT).sum(axis=1, dtype=np.float32)
    return out, aux


# revision 39
# speedup vs baseline: 1.9047x; 1.0177x over previous
"""MoE (top-2 of 32 experts, 512->512) on 8 NeuronCores, expert-parallel.

Strategy (full-I/O contract: kernel() receives full inputs, returns full output):
  - Host computes the small selector (softmax gate + top-k) in fp32 numpy,
    mirroring the reference op-for-op, and performs the "all-to-all dispatch":
    tokens are grouped by expert into capacity-padded batches (the host-side
    sharding step of the expert-parallel layout).
  - Experts are sharded 4-per-core across the 8 cores.  Each core runs a Bass
    kernel: for each of its experts, Y = relu(X_e @ We[e] + be[e]) * gate_w,
    with the per-token gate weight fused into the relu via the ScalarE
    activation (scale is per-partition; gate weights are >= 0 so
    w*relu(z) == relu(w*z)).
  - Host combines: out[token] = sum of its k slot rows (weighted on device).

Precision modes for the expert GEMMs (MODE):
  - "f32"  : exact fp32 matmuls (PE runs them as 2 half-rate passes).
  - "f16x3": x and W split into fp16 hi+lo; y = xh@wh + xh@wl + xl@wh.
             fp16 products are exact in fp32 (11+11 mantissa bits < 24), so
             the only extra error is the dropped xl@wl term (~2^-22 relative)
             - fp32-equivalent accuracy at 3/4 the PE cost of "f32".
  - "f32r" : single-pass relaxed-precision fp32 (tf32-like, ~1e-4 rel err).
  - "bf16" : single-pass bf16 (~1e-3 rel err).

All tensors are pre-swizzled on the host into the exact SBUF layout so every
device DMA is a contiguous copy.  The device kernel is compiled once per
(capacity, mode, has_bias) and cached.
"""

import numpy as np
from contextlib import ExitStack

B, NIN, NOUT, E, NCORES = 8192, 512, 512, 32, 8
EPC = E // NCORES  # experts per core
KCH = NIN // 128   # contraction chunks of 128

MODE = "f16x3"

_CACHE = {}


def _mode_spec(mode):
    import concourse.mybir as mybir
    import ml_dtypes
    if mode == "f32":
        return dict(dt=mybir.dt.float32, npdt=np.float32, nsplit=1,
                    passes=[(0, 0)])
    if mode == "f32r":
        return dict(dt=mybir.dt.float32r, npdt=np.float32, nsplit=1,
                    passes=[(0, 0)])
    if mode == "bf16":
        return dict(dt=mybir.dt.bfloat16, npdt=ml_dtypes.bfloat16, nsplit=1,
                    passes=[(0, 0)])
    if mode == "f16x3":
        return dict(dt=mybir.dt.float16, npdt=np.float16, nsplit=2,
                    passes=[(0, 0), (0, 1), (1, 0)])
    raise ValueError(mode)


def _build(caps, mode, has_bias):
    """Build + compile the per-core Bass program.  caps[j] = number of
    128-token blocks for expert slot j (same structure on every core; the
    host assigns its busiest expert to slot 0 etc. so capacity is not
    wasted padding every expert to the global max)."""
    import concourse.mybir as mybir
    import concourse.tile as tile
    from concourse import bacc

    spec = _mode_spec(mode)
    mmdt = spec["dt"]
    nsplit = spec["nsplit"]
    passes = spec["passes"]
    Cj = [c * 128 for c in caps]           # tokens per slot
    CT = sum(Cj)                           # tokens per core
    NBLK = sum(caps)
    ROFF = np.concatenate([[0], np.cumsum(Cj)]).astype(int)   # token offsets
    BOFF = np.concatenate([[0], np.cumsum(caps)]).astype(int)  # block offsets
    XOFF = [KCH * r for r in ROFF]         # x free-elem offsets per slot
    XF = KCH * CT           # x free elems per split: [p][slot][kc][c]
    WF = EPC * KCH * NOUT   # w free elems per split: [p][slot][kc][n]

    nc = bacc.Bacc("TRN2", target_bir_lowering=False, debug=False,
                   num_devices=NCORES)
    # hi/lo splits are stacked along the free axis of ONE tensor so a single
    # DMA moves both (fewer DMA triggers - they serialize on the HWDGE ring)
    xd = nc.dram_tensor("xd", [128, nsplit * XF], mmdt,
                        kind="ExternalInput").ap()
    wd = nc.dram_tensor("wd", [128, nsplit * WF], mmdt,
                        kind="ExternalInput").ap()
    cw = nc.dram_tensor("cw", [128, NBLK], mybir.dt.float32,
                        kind="ExternalInput").ap()
    if has_bias:
        be = nc.dram_tensor("be", [1, EPC * NOUT], mybir.dt.float32,
                            kind="ExternalInput").ap()
    out = nc.dram_tensor("out", [CT, NOUT], mybir.dt.float32,
                         kind="ExternalOutput").ap()

    with tile.TileContext(nc) as tc, ExitStack() as ctx:
        xpool = ctx.enter_context(tc.tile_pool(name="x", bufs=1))
        wpool = ctx.enter_context(tc.tile_pool(name="w", bufs=1))
        spool = ctx.enter_context(tc.tile_pool(name="s", bufs=1))
        opool = ctx.enter_context(tc.tile_pool(name="o", bufs=1))
        pspool = ctx.enter_context(tc.tile_pool(name="ps", bufs=1,
                                                space="PSUM"))

        x_all = xpool.tile([128, nsplit * XF], mmdt, name="x_all")
        w_all = wpool.tile([128, nsplit * WF], mmdt, name="w_all")
        x_sb = [x_all[:, s * XF:(s + 1) * XF] for s in range(nsplit)]
        w_sb = [w_all[:, s * WF:(s + 1) * WF] for s in range(nsplit)]

        # cw (and bias) first: tiny, and every ACT epilogue needs it - behind
        # the bulk inputs on the FIFO ring it would stall all PSUM recycling.
        cw_sb = spool.tile([128, NBLK], mybir.dt.float32)
        nc.sync.dma_start(cw_sb[:], cw)
        if has_bias:
            bias_sb = spool.tile([1, EPC * NOUT], mybir.dt.float32)
            nc.sync.dma_start(bias_sb[:], be)
            ones_sb = spool.tile([1, 128], mybir.dt.float32)
            nc.vector.memset(ones_sb[:], 1.0)

        # Input DMAs per (expert, kc-half): one DMA covers hi+lo (2 strided
        # regions).  Inputs ride the SP HWDGE ring; outputs ride the ACT ring
        # to avoid head-of-line blocking between the two streams.
        xv = xd.rearrange("p (s f) -> p s f", s=nsplit)
        wv = wd.rearrange("p (s f) -> p s f", s=nsplit)
        xav = x_all[:].rearrange("p (s f) -> p s f", s=nsplit)
        wav = w_all[:].rearrange("p (s f) -> p s f", s=nsplit)
        for j in range(EPC):
            if caps[j] == 0:
                continue
            # finer chunks for slot 0 so the first matmul starts sooner
            KH = 1 if j == 0 else 2
            for kh in range(0, KCH, KH):
                xl_ = XOFF[j] + kh * Cj[j]
                xr_ = XOFF[j] + (kh + KH) * Cj[j]
                wl_ = (j * KCH + kh) * NOUT
                wr_ = (j * KCH + kh + KH) * NOUT
                nc.sync.dma_start(xav[:, :, xl_:xr_], xv[:, :, xl_:xr_])
                nc.sync.dma_start(wav[:, :, wl_:wr_], wv[:, :, wl_:wr_])

        out_sb = opool.tile([128, NBLK * NOUT], mybir.dt.float32)

        # PE warmup: dependency-free dummy matmuls ramp the PE clock to full
        # speed while the first input DMAs are still in flight.
        wu_sb = spool.tile([128, NOUT], mmdt)
        nc.vector.memset(wu_sb[:], 0.0)
        wu_ps = pspool.tile([128, NOUT], mybir.dt.float32, tag="ps_wu")
        for _ in range(6):
            nc.tensor.matmul(wu_ps[:], lhsT=wu_sb[:, :128], rhs=wu_sb[:],
                             start=True, stop=True)

        npass = len(passes)
        nmm = KCH * npass + (1 if has_bias else 0)
        for j in range(EPC):
            nblk = caps[j]
            if nblk == 0:
                continue
            # Slot 0 runs k-chunk outer (each block's PSUM bank stays live
            # across the kc sweep) so the first matmuls only need the first
            # (x, w) k-chunk.  Later slots have all data resident by the
            # time they run, so they go block-major, which spreads the ACT +
            # out-DMA epilogues instead of bunching them at the slot end.
            pss = [pspool.tile([128, NOUT], mybir.dt.float32,
                               name=f"ps_{j}_{t}", tag=f"ps{t}")
                   for t in range(nblk)]
            if j == 0:
                mm_order = [(kc, t) for kc in range(KCH)
                            for t in range(nblk)]
            else:
                mm_order = [(kc, t) for t in range(nblk)
                            for kc in range(KCH)]
            for (kc, t) in mm_order:
                xoff = XOFF[j] + kc * Cj[j]
                woff = (j * KCH + kc) * NOUT
                for p, (sx, sw) in enumerate(passes):
                    m = kc * npass + p + 1
                    nc.tensor.matmul(
                        pss[t][:],
                        lhsT=x_sb[sx][:, xoff + t * 128:
                                      xoff + (t + 1) * 128],
                        rhs=w_sb[sw][:, woff:woff + NOUT],
                        start=(m == 1), stop=(m == nmm))
            for t in range(nblk):
                blk = BOFF[j] + t
                if has_bias:
                    nc.tensor.matmul(
                        pss[t][:], lhsT=ones_sb[:],
                        rhs=bias_sb[:, j * NOUT:(j + 1) * NOUT],
                        start=False, stop=True)
                nc.scalar.activation(
                    out_sb[:, blk * NOUT:(blk + 1) * NOUT], pss[t][:],
                    mybir.ActivationFunctionType.Relu,
                    scale=cw_sb[:, blk:blk + 1])
            # out DMAs per slot-half (fewer triggers); per-block for the
            # last slot so the kernel tail is short
            if j < EPC - 1:
                tranges = ((0, nblk - nblk // 2), (nblk - nblk // 2, nblk))
            else:
                tranges = tuple((t, t + 1) for t in range(nblk))
            for (t0, t1) in tranges:
                nc.scalar.dma_start(
                    out[ROFF[j] + t0 * 128:ROFF[j] + t1 * 128, :]
                    .rearrange("(t p) n -> p t n", p=128),
                    out_sb[:].rearrange("p (b n) -> p b n", n=NOUT)
                    [:, BOFF[j] + t0:BOFF[j] + t1, :])

    nc.compile()
    return nc


def _make_runner(nc):
    """One-time jit of the 8-core SPMD executable (mirrors
    bass2jax.run_bass_via_pjrt, cached so repeat calls skip retracing)."""
    import jax
    import jax.core
    import numpy as _np
    from jax.sharding import Mesh, PartitionSpec
    from jax.experimental.shard_map import shard_map
    from concourse import bass2jax, mybir

    bass2jax.install_neuronx_cc_hook()

    partition_name = (nc.partition_id_tensor.name
                      if nc.partition_id_tensor else None)
    in_names, out_names, out_avals, zero_shapes = [], [], [], []
    for alloc in nc.m.functions[0].allocations:
        if not isinstance(alloc, mybir.MemoryLocationSet):
            continue
        name = alloc.memorylocations[0].name
        if alloc.kind == "ExternalInput":
            if name != partition_name:
                in_names.append(name)
        elif alloc.kind == "ExternalOutput":
            out_names.append(name)
            shape = tuple(alloc.tensor_shape)
            dt = mybir.dt.np(alloc.dtype)
            out_avals.append(jax.core.ShapedArray(shape, dt))
            zero_shapes.append((shape, dt))
    n_params = len(in_names)
    all_names = in_names + out_names
    if partition_name is not None:
        all_names = all_names + [partition_name]

    def _body(*args):
        operands = list(args)
        if partition_name is not None:
            operands.append(bass2jax.partition_id_tensor())
        outs = bass2jax._bass_exec_p.bind(
            *operands,
            out_avals=tuple(out_avals),
            in_names=tuple(all_names),
            out_names=tuple(out_names),
            lowering_input_output_aliases=(),
            sim_require_finite=True,
            sim_require_nnan=True,
            nc=nc,
        )
        return tuple(outs)

    devices = jax.devices()[:NCORES]
    mesh = Mesh(_np.asarray(devices), ("core",))
    n_outs = len(out_names)
    specs = (PartitionSpec("core"),) * (n_params + n_outs)
    donate = tuple(range(n_params, n_params + n_outs))
    sharded = jax.jit(
        shard_map(_body, mesh=mesh, in_specs=specs,
                  out_specs=(PartitionSpec("core"),) * n_outs,
                  check_rep=False),
        donate_argnums=donate, keep_unused=True)

    def run(feeds):
        """feeds: dict name -> full concatenated array [NCORES*dim0, ...].
        Returns dict name -> full concatenated output array."""
        concat_in = [feeds[name] for name in in_names]
        concat_zeros = [
            _np.zeros((NCORES * s[0],) + tuple(s[1:]), dt)
            for (s, dt) in zero_shapes
        ]
        out_arrs = sharded(*concat_in, *concat_zeros)
        return {name: _np.asarray(out_arrs[i])
                for i, name in enumerate(out_names)}

    # exposed for benchmarking (test.py)
    run._sharded = sharded
    run._in_names = in_names
    run._zero_shapes = zero_shapes
    return run


def _get_runner(caps, mode, has_bias):
    key = (caps, mode, has_bias)
    if key not in _CACHE:
        nc = _build(caps, mode, has_bias)
        _CACHE[key] = (nc, _make_runner(nc))
    return _CACHE[key]


def _route(x, Wg, bg, k):
    """Replicates the reference selector in fp32: softmax gate, top-k
    (stable, ties to lower index like jax.lax.top_k), aux loss."""
    logits = x @ Wg + bg
    m = logits.max(-1, keepdims=True)
    p = np.exp(logits - m)
    gate = p / p.sum(-1, keepdims=True)
    idx = np.argsort(-gate, axis=-1, kind="stable")[:, :k]      # [B, k]
    vals = np.take_along_axis(gate, idx, axis=-1)               # [B, k]
    row_sum = gate.sum(-1)
    aux = (np.var(row_sum) / (np.mean(row_sum) ** 2 + np.float32(1e-10)))
    return idx, vals, np.float32(aux)


def _split_into(dst, view, spec, F):
    """Write `view` [NCORES, 128, F] (fp32, any strides) into dst
    [NCORES*128, nsplit*F] as hi (and lo residual for split modes)."""
    d = dst.reshape(NCORES, 128, -1)
    d[:, :, :F] = view                      # cast fp32 -> device dtype
    if spec["nsplit"] == 2:
        d[:, :, F:] = view - d[:, :, :F].astype(np.float32)


def _prepare(x, Wg, bg, We, be, k, mode):
    """Route + dispatch: returns (caps, has_bias, feeds, dest, inv, aux)."""
    spec = _mode_spec(mode)
    idx, vals, aux = _route(x, Wg, bg, k)

    ef = idx.ravel()
    wf = vals.ravel()
    tf = np.repeat(np.arange(B), k)
    order = np.argsort(ef, kind="stable")
    counts = np.bincount(ef, minlength=E)

    # Load-sorted slot assignment: expert with load-rank r goes to core r%8,
    # slot r//8; slot j's capacity is the max block count in rank octile j,
    # so every core compiles to the same (caps) block structure.
    blocks = np.maximum(1, -(-counts // 128))
    rank_order = np.argsort(-counts, kind="stable")      # expert ids by load
    caps = tuple(int(blocks[rank_order[NCORES * j]]) for j in range(EPC))
    Cj = np.array([c * 128 for c in caps])
    CT = int(Cj.sum())
    NBLK = sum(caps)
    ROFF = np.concatenate([[0], np.cumsum(Cj)]).astype(np.int64)

    core_of = np.empty(E, dtype=np.int64)
    slot_of = np.empty(E, dtype=np.int64)
    core_of[rank_order] = np.arange(E) % NCORES
    slot_of[rank_order] = np.arange(E) // NCORES

    starts = np.zeros(E, dtype=np.int64)
    starts[1:] = np.cumsum(counts)[:-1]
    base = core_of * CT + ROFF[slot_of]                  # per-expert row base
    es = ef[order]
    dest = base[es] + (np.arange(B * k) - starts[es])    # unique global rows
    inv = np.argsort(order, kind="stable")

    Xg = np.zeros((NCORES * CT, NIN), dtype=np.float32)
    Xg[dest] = x[tf[order]]
    cwg = np.zeros(NCORES * CT, dtype=np.float32)
    cwg[dest] = wf[order]

    npdt = spec["npdt"]
    ns = spec["nsplit"]
    XF = KCH * CT
    WF = EPC * KCH * NOUT

    # swizzle straight into the concat-ready device feeds
    # x: [core][p][slot][kc][c] with per-slot capacities
    xsw = np.empty((NCORES, 128, XF), np.float32)
    Xg3 = Xg.reshape(NCORES, CT, NIN)
    for j in range(EPC):
        xsw[:, :, KCH * ROFF[j]:KCH * ROFF[j + 1]] = (
            Xg3[:, ROFF[j]:ROFF[j + 1], :]
            .reshape(NCORES, Cj[j], KCH, 128).transpose(0, 3, 2, 1)
            .reshape(NCORES, 128, KCH * Cj[j]))
    xd = np.empty((NCORES * 128, ns * XF), npdt)
    _split_into(xd, xsw, spec, XF)

    # w: [core][p][slot][kc][n]; core c's slot j holds expert rank_order[8j+c]
    eid = rank_order.reshape(EPC, NCORES).T              # [core, slot]
    wd = np.empty((NCORES * 128, ns * WF), npdt)
    _split_into(wd, We[eid].reshape(NCORES, EPC, KCH, 128, NOUT)
                .transpose(0, 3, 1, 2, 4).reshape(NCORES, 128, WF), spec, WF)

    # cw: [core][p][global block]
    csw = np.empty((NCORES, 128, NBLK), np.float32)
    cw3 = cwg.reshape(NCORES, CT)
    boff = 0
    for j in range(EPC):
        csw[:, :, boff:boff + caps[j]] = (
            cw3[:, ROFF[j]:ROFF[j + 1]]
            .reshape(NCORES, caps[j], 128).transpose(0, 2, 1))
        boff += caps[j]

    has_bias = bool(np.any(be))
    feeds = {"cw": csw.reshape(NCORES * 128, NBLK), "xd": xd, "wd": wd}
    if has_bias:
        feeds["be"] = np.ascontiguousarray(
            be[eid].reshape(NCORES, EPC * NOUT))   # [core][1, e*n] rows
    return caps, has_bias, feeds, dest, inv, aux


def kernel(x, Wg, bg, We, be, k):
    x = np.ascontiguousarray(np.asarray(x, dtype=np.float32))
    Wg = np.asarray(Wg, dtype=np.float32)
    bg = np.asarray(bg, dtype=np.float32)
    We = np.ascontiguousarray(np.asarray(We, dtype=np.float32))
    be = np.ascontiguousarray(np.asarray(be, dtype=np.float32))
    k = int(k)

    caps, has_bias, feeds, dest, inv, aux = _prepare(x, Wg, bg, We, be, k,
                                                     MODE)
    _, run = _get_runner(caps, MODE, has_bias)
    results = run(feeds)

    Y = results["out"]                                  # [E*C, NOUT] global
    slot_rows = Y[dest[inv]]                            # token-major slots
    out = slot_rows.reshape(B, k, NOUT).sum(axis=1, dtype=np.float32)
    return out, aux


# revision 52
# speedup vs baseline: 1.9221x; 1.0091x over previous
"""MoE (top-2 of 32 experts, 512->512) on 8 NeuronCores, expert-parallel.

Strategy (full-I/O contract: kernel() receives full inputs, returns full output):
  - Host computes the small selector (softmax gate + top-k) in fp32 numpy,
    mirroring the reference op-for-op, and performs the "all-to-all dispatch":
    tokens are grouped by expert into capacity-padded batches (the host-side
    sharding step of the expert-parallel layout).
  - Experts are sharded 4-per-core across the 8 cores.  Each core runs a Bass
    kernel: for each of its experts, Y = relu(X_e @ We[e] + be[e]) * gate_w,
    with the per-token gate weight fused into the relu via the ScalarE
    activation (scale is per-partition; gate weights are >= 0 so
    w*relu(z) == relu(w*z)).
  - Host combines: out[token] = sum of its k slot rows (weighted on device).

Precision modes for the expert GEMMs (MODE):
  - "f32"  : exact fp32 matmuls (PE runs them as 2 half-rate passes).
  - "f16x3": x and W split into fp16 hi+lo; y = xh@wh + xh@wl + xl@wh.
             fp16 products are exact in fp32 (11+11 mantissa bits < 24), so
             the only extra error is the dropped xl@wl term (~2^-22 relative)
             - fp32-equivalent accuracy at 3/4 the PE cost of "f32".
  - "f32r" : single-pass relaxed-precision fp32 (tf32-like, ~1e-4 rel err).
  - "bf16" : single-pass bf16 (~1e-3 rel err).

All tensors are pre-swizzled on the host into the exact SBUF layout so every
device DMA is a contiguous copy.  The device kernel is compiled once per
(capacity, mode, has_bias) and cached.
"""

import numpy as np
from contextlib import ExitStack

B, NIN, NOUT, E, NCORES = 8192, 512, 512, 32, 8
EPC = E // NCORES  # experts per core
KCH = NIN // 128   # contraction chunks of 128

MODE = "f16x3"

_CACHE = {}


def _mode_spec(mode):
    import concourse.mybir as mybir
    import ml_dtypes
    if mode == "f32":
        return dict(dt=mybir.dt.float32, npdt=np.float32, nsplit=1,
                    passes=[(0, 0)])
    if mode == "f32r":
        return dict(dt=mybir.dt.float32r, npdt=np.float32, nsplit=1,
                    passes=[(0, 0)])
    if mode == "bf16":
        return dict(dt=mybir.dt.bfloat16, npdt=ml_dtypes.bfloat16, nsplit=1,
                    passes=[(0, 0)])
    if mode == "f16x3":
        return dict(dt=mybir.dt.float16, npdt=np.float16, nsplit=2,
                    passes=[(0, 0), (0, 1), (1, 0)])
    raise ValueError(mode)


def _build(caps, mode, has_bias):
    """Build + compile the per-core Bass program.  caps[j] = number of
    128-token blocks for expert slot j (same structure on every core; the
    host assigns its busiest expert to slot 0 etc. so capacity is not
    wasted padding every expert to the global max)."""
    import concourse.mybir as mybir
    import concourse.tile as tile
    from concourse import bacc

    spec = _mode_spec(mode)
    mmdt = spec["dt"]
    nsplit = spec["nsplit"]
    passes = spec["passes"]
    Cj = [c * 128 for c in caps]           # tokens per slot
    CT = sum(Cj)                           # tokens per core
    NBLK = sum(caps)
    ROFF = np.concatenate([[0], np.cumsum(Cj)]).astype(int)   # token offsets
    BOFF = np.concatenate([[0], np.cumsum(caps)]).astype(int)  # block offsets
    XOFF = [KCH * r for r in ROFF]         # x free-elem offsets per slot
    XF = KCH * CT           # x free elems per split: [p][slot][kc][c]
    WF = EPC * KCH * NOUT   # w free elems per split: [p][slot][kc][n]

    nc = bacc.Bacc("TRN2", target_bir_lowering=False, debug=False,
                   num_devices=NCORES)
    # hi/lo splits are stacked along the free axis of ONE tensor so a single
    # DMA moves both (fewer DMA triggers - they serialize on the HWDGE ring)
    xd = nc.dram_tensor("xd", [128, nsplit * XF], mmdt,
                        kind="ExternalInput").ap()
    wd = nc.dram_tensor("wd", [128, nsplit * WF], mmdt,
                        kind="ExternalInput").ap()
    cw = nc.dram_tensor("cw", [128, NBLK], mybir.dt.float32,
                        kind="ExternalInput").ap()
    if has_bias:
        be = nc.dram_tensor("be", [1, EPC * NOUT], mybir.dt.float32,
                            kind="ExternalInput").ap()
    out = nc.dram_tensor("out", [CT, NOUT], mybir.dt.float32,
                         kind="ExternalOutput").ap()

    with tile.TileContext(nc) as tc, ExitStack() as ctx:
        xpool = ctx.enter_context(tc.tile_pool(name="x", bufs=1))
        wpool = ctx.enter_context(tc.tile_pool(name="w", bufs=1))
        spool = ctx.enter_context(tc.tile_pool(name="s", bufs=1))
        opool = ctx.enter_context(tc.tile_pool(name="o", bufs=1))
        pspool = ctx.enter_context(tc.tile_pool(name="ps", bufs=1,
                                                space="PSUM"))

        x_all = xpool.tile([128, nsplit * XF], mmdt, name="x_all")
        w_all = wpool.tile([128, nsplit * WF], mmdt, name="w_all")
        x_sb = [x_all[:, s * XF:(s + 1) * XF] for s in range(nsplit)]
        w_sb = [w_all[:, s * WF:(s + 1) * WF] for s in range(nsplit)]

        # cw (and bias) first: tiny, and every ACT epilogue needs it - behind
        # the bulk inputs on the FIFO ring it would stall all PSUM recycling.
        cw_sb = spool.tile([128, NBLK], mybir.dt.float32)
        nc.sync.dma_start(cw_sb[:], cw)
        if has_bias:
            bias_sb = spool.tile([1, EPC * NOUT], mybir.dt.float32)
            nc.sync.dma_start(bias_sb[:], be)
            ones_sb = spool.tile([1, 128], mybir.dt.float32)
            nc.vector.memset(ones_sb[:], 1.0)

        # Input DMAs per (expert, kc-half): one DMA covers hi+lo (2 strided
        # regions).  Inputs ride the SP HWDGE ring; outputs ride the ACT ring
        # to avoid head-of-line blocking between the two streams.
        xv = xd.rearrange("p (s f) -> p s f", s=nsplit)
        wv = wd.rearrange("p (s f) -> p s f", s=nsplit)
        xav = x_all[:].rearrange("p (s f) -> p s f", s=nsplit)
        wav = w_all[:].rearrange("p (s f) -> p s f", s=nsplit)
        for j in range(EPC):
            if caps[j] == 0:
                continue
            # one DMA covers hi+lo; finer chunks for slot 0 so the first
            # matmul starts sooner.  Slot 0's first k-chunk additionally
            # ships hi before lo, in the order the (kc, pass, t) matmul
            # stream consumes it: xh, wh (pass 1), wl (pass 2), xl (pass 3).
            if j == 0 and nsplit == 2:
                for (s, a_, v_, l_, r_) in (
                        (0, xav, xv, XOFF[0], XOFF[0] + Cj[0]),
                        (0, wav, wv, 0, NOUT),
                        (1, wav, wv, 0, NOUT),
                        (1, xav, xv, XOFF[0], XOFF[0] + Cj[0])):
                    nc.sync.dma_start(a_[:, s:s + 1, l_:r_],
                                      v_[:, s:s + 1, l_:r_])
            KH = 1 if j == 0 else 2
            for kh in range(KH if j == 0 else 0, KCH, KH):
                xl_ = XOFF[j] + kh * Cj[j]
                xr_ = XOFF[j] + (kh + KH) * Cj[j]
                wl_ = (j * KCH + kh) * NOUT
                wr_ = (j * KCH + kh + KH) * NOUT
                nc.sync.dma_start(xav[:, :, xl_:xr_], xv[:, :, xl_:xr_])
                nc.sync.dma_start(wav[:, :, wl_:wr_], wv[:, :, wl_:wr_])

        out_sb = opool.tile([128, NBLK * NOUT], mybir.dt.float32)

        # PE warmup: dependency-free dummy matmuls ramp the PE clock to full
        # speed while the first input DMAs are still in flight.
        wu_sb = spool.tile([128, NOUT], mmdt)
        nc.vector.memset(wu_sb[:], 0.0)
        wu_ps = pspool.tile([128, NOUT], mybir.dt.float32, tag="ps_wu")
        for _ in range(4):
            nc.tensor.matmul(wu_ps[:], lhsT=wu_sb[:, :128], rhs=wu_sb[:],
                             start=True, stop=True)

        npass = len(passes)
        nmm = KCH * npass + (1 if has_bias else 0)
        for j in range(EPC):
            nblk = caps[j]
            if nblk == 0:
                continue
            # Slot 0 runs k-chunk outer (each block's PSUM bank stays live
            # across the kc sweep) so the first matmuls only need the first
            # (x, w) k-chunk.  Later slots have all data resident by the
            # time they run, so they go block-major, which spreads the ACT +
            # out-DMA epilogues instead of bunching them at the slot end.
            pss = [pspool.tile([128, NOUT], mybir.dt.float32,
                               name=f"ps_{j}_{t}", tag=f"ps{t}")
                   for t in range(nblk)]
            if j == 0:
                # k-chunk outer, pass-mid: the first 5 matmuls need only
                # xh+wh of kc0, matching the hi-first DMA order above
                mm_order = [(p, kc, t) for kc in range(KCH)
                            for p in range(npass) for t in range(nblk)]
            else:
                mm_order = [(p, kc, t) for t in range(nblk)
                            for kc in range(KCH) for p in range(npass)]

            def acc_idx(p, kc):
                return kc * npass + p + 1
            for (p, kc, t) in mm_order:
                xoff = XOFF[j] + kc * Cj[j]
                woff = (j * KCH + kc) * NOUT
                sx, sw = passes[p]
                m = acc_idx(p, kc)
                nc.tensor.matmul(
                    pss[t][:],
                    lhsT=x_sb[sx][:, xoff + t * 128:
                                  xoff + (t + 1) * 128],
                    rhs=w_sb[sw][:, woff:woff + NOUT],
                    start=(m == 1), stop=(m == nmm))
            for t in range(nblk):
                blk = BOFF[j] + t
                if has_bias:
                    nc.tensor.matmul(
                        pss[t][:], lhsT=ones_sb[:],
                        rhs=bias_sb[:, j * NOUT:(j + 1) * NOUT],
                        start=False, stop=True)
                if j == EPC - 1 and t == nblk - 1:
                    # final block: halve the epilogue so its ACT and its
                    # out-DMA overlap (shortens the kernel tail); the DMAs
                    # ride the long-idle SP ring
                    for h in (0, NOUT // 2):
                        nc.scalar.activation(
                            out_sb[:, blk * NOUT + h:
                                   blk * NOUT + h + NOUT // 2],
                            pss[t][:, h:h + NOUT // 2],
                            mybir.ActivationFunctionType.Relu,
                            scale=cw_sb[:, blk:blk + 1])
                        nc.sync.dma_start(
                            out[ROFF[j] + t * 128:ROFF[j] + (t + 1) * 128,
                                h:h + NOUT // 2],
                            out_sb[:, blk * NOUT + h:
                                   blk * NOUT + h + NOUT // 2])
                    continue
                nc.scalar.activation(
                    out_sb[:, blk * NOUT:(blk + 1) * NOUT], pss[t][:],
                    mybir.ActivationFunctionType.Relu,
                    scale=cw_sb[:, blk:blk + 1])
            # out DMAs per slot-half (fewer triggers); per-block for the
            # last slot so the kernel tail is short
            if j < EPC - 1:
                tranges = ((0, nblk - nblk // 2), (nblk - nblk // 2, nblk))
            else:
                # final block's out-DMA was emitted inline above
                tranges = tuple((t, t + 1) for t in range(nblk - 1))
            # Early slots' outputs ride the ACT HWDGE ring (SP is busy with
            # inputs); late slots ride the by-then-idle SP ring, so the tail
            # isn't serialized behind ACT-ring trigger costs.
            out_eng = nc.scalar if j < EPC // 2 else nc.sync
            for (t0, t1) in tranges:
                out_eng.dma_start(
                    out[ROFF[j] + t0 * 128:ROFF[j] + t1 * 128, :]
                    .rearrange("(t p) n -> p t n", p=128),
                    out_sb[:].rearrange("p (b n) -> p b n", n=NOUT)
                    [:, BOFF[j] + t0:BOFF[j] + t1, :])

    nc.compile()
    return nc


def _make_runner(nc):
    """One-time jit of the 8-core SPMD executable (mirrors
    bass2jax.run_bass_via_pjrt, cached so repeat calls skip retracing)."""
    import jax
    import jax.core
    import numpy as _np
    from jax.sharding import Mesh, PartitionSpec
    from jax.experimental.shard_map import shard_map
    from concourse import bass2jax, mybir

    bass2jax.install_neuronx_cc_hook()

    partition_name = (nc.partition_id_tensor.name
                      if nc.partition_id_tensor else None)
    in_names, out_names, out_avals, zero_shapes = [], [], [], []
    for alloc in nc.m.functions[0].allocations:
        if not isinstance(alloc, mybir.MemoryLocationSet):
            continue
        name = alloc.memorylocations[0].name
        if alloc.kind == "ExternalInput":
            if name != partition_name:
                in_names.append(name)
        elif alloc.kind == "ExternalOutput":
            out_names.append(name)
            shape = tuple(alloc.tensor_shape)
            dt = mybir.dt.np(alloc.dtype)
            out_avals.append(jax.core.ShapedArray(shape, dt))
            zero_shapes.append((shape, dt))
    n_params = len(in_names)
    all_names = in_names + out_names
    if partition_name is not None:
        all_names = all_names + [partition_name]

    def _body(*args):
        operands = list(args)
        if partition_name is not None:
            operands.append(bass2jax.partition_id_tensor())
        outs = bass2jax._bass_exec_p.bind(
            *operands,
            out_avals=tuple(out_avals),
            in_names=tuple(all_names),
            out_names=tuple(out_names),
            lowering_input_output_aliases=(),
            sim_require_finite=True,
            sim_require_nnan=True,
            nc=nc,
        )
        return tuple(outs)

    devices = jax.devices()[:NCORES]
    mesh = Mesh(_np.asarray(devices), ("core",))
    n_outs = len(out_names)
    specs = (PartitionSpec("core"),) * (n_params + n_outs)
    donate = tuple(range(n_params, n_params + n_outs))
    sharded = jax.jit(
        shard_map(_body, mesh=mesh, in_specs=specs,
                  out_specs=(PartitionSpec("core"),) * n_outs,
                  check_rep=False),
        donate_argnums=donate, keep_unused=True)

    def run(feeds):
        """feeds: dict name -> full concatenated array [NCORES*dim0, ...].
        Returns dict name -> full concatenated output array."""
        concat_in = [feeds[name] for name in in_names]
        concat_zeros = [
            _np.zeros((NCORES * s[0],) + tuple(s[1:]), dt)
            for (s, dt) in zero_shapes
        ]
        out_arrs = sharded(*concat_in, *concat_zeros)
        return {name: _np.asarray(out_arrs[i])
                for i, name in enumerate(out_names)}

    # exposed for benchmarking (test.py)
    run._sharded = sharded
    run._in_names = in_names
    run._zero_shapes = zero_shapes
    return run


def _get_runner(caps, mode, has_bias):
    key = (caps, mode, has_bias)
    if key not in _CACHE:
        nc = _build(caps, mode, has_bias)
        _CACHE[key] = (nc, _make_runner(nc))
    return _CACHE[key]


def _route(x, Wg, bg, k):
    """Replicates the reference selector in fp32: softmax gate, top-k
    (stable, ties to lower index like jax.lax.top_k), aux loss."""
    logits = x @ Wg + bg
    m = logits.max(-1, keepdims=True)
    p = np.exp(logits - m)
    gate = p / p.sum(-1, keepdims=True)
    idx = np.argsort(-gate, axis=-1, kind="stable")[:, :k]      # [B, k]
    vals = np.take_along_axis(gate, idx, axis=-1)               # [B, k]
    row_sum = gate.sum(-1)
    aux = (np.var(row_sum) / (np.mean(row_sum) ** 2 + np.float32(1e-10)))
    return idx, vals, np.float32(aux)


def _split_into(dst, view, spec, F):
    """Write `view` [NCORES, 128, F] (fp32, any strides) into dst
    [NCORES*128, nsplit*F] as hi (and lo residual for split modes)."""
    d = dst.reshape(NCORES, 128, -1)
    d[:, :, :F] = view                      # cast fp32 -> device dtype
    if spec["nsplit"] == 2:
        d[:, :, F:] = view - d[:, :, :F].astype(np.float32)


def _prepare(x, Wg, bg, We, be, k, mode):
    """Route + dispatch: returns (caps, has_bias, feeds, dest, inv, aux)."""
    spec = _mode_spec(mode)
    idx, vals, aux = _route(x, Wg, bg, k)

    ef = idx.ravel()
    wf = vals.ravel()
    tf = np.repeat(np.arange(B), k)
    order = np.argsort(ef, kind="stable")
    counts = np.bincount(ef, minlength=E)

    # Load-sorted slot assignment: expert with load-rank r goes to core r%8,
    # slot r//8; slot j's capacity is the max block count in rank octile j,
    # so every core compiles to the same (caps) block structure.
    blocks = np.maximum(1, -(-counts // 128))
    rank_order = np.argsort(-counts, kind="stable")      # expert ids by load
    caps = tuple(int(blocks[rank_order[NCORES * j]]) for j in range(EPC))
    Cj = np.array([c * 128 for c in caps])
    CT = int(Cj.sum())
    NBLK = sum(caps)
    ROFF = np.concatenate([[0], np.cumsum(Cj)]).astype(np.int64)

    core_of = np.empty(E, dtype=np.int64)
    slot_of = np.empty(E, dtype=np.int64)
    core_of[rank_order] = np.arange(E) % NCORES
    slot_of[rank_order] = np.arange(E) // NCORES

    starts = np.zeros(E, dtype=np.int64)
    starts[1:] = np.cumsum(counts)[:-1]
    base = core_of * CT + ROFF[slot_of]                  # per-expert row base
    es = ef[order]
    dest = base[es] + (np.arange(B * k) - starts[es])    # unique global rows
    inv = np.argsort(order, kind="stable")

    Xg = np.zeros((NCORES * CT, NIN), dtype=np.float32)
    Xg[dest] = x[tf[order]]
    cwg = np.zeros(NCORES * CT, dtype=np.float32)
    cwg[dest] = wf[order]

    npdt = spec["npdt"]
    ns = spec["nsplit"]
    XF = KCH * CT
    WF = EPC * KCH * NOUT

    # swizzle straight into the concat-ready device feeds
    # x: [core][p][slot][kc][c] with per-slot capacities
    xsw = np.empty((NCORES, 128, XF), np.float32)
    Xg3 = Xg.reshape(NCORES, CT, NIN)
    for j in range(EPC):
        xsw[:, :, KCH * ROFF[j]:KCH * ROFF[j + 1]] = (
            Xg3[:, ROFF[j]:ROFF[j + 1], :]
            .reshape(NCORES, Cj[j], KCH, 128).transpose(0, 3, 2, 1)
            .reshape(NCORES, 128, KCH * Cj[j]))
    xd = np.empty((NCORES * 128, ns * XF), npdt)
    _split_into(xd, xsw, spec, XF)

    # w: [core][p][slot][kc][n]; core c's slot j holds expert rank_order[8j+c]
    eid = rank_order.reshape(EPC, NCORES).T              # [core, slot]
    wd = np.empty((NCORES * 128, ns * WF), npdt)
    _split_into(wd, We[eid].reshape(NCORES, EPC, KCH, 128, NOUT)
                .transpose(0, 3, 1, 2, 4).reshape(NCORES, 128, WF), spec, WF)

    # cw: [core][p][global block]
    csw = np.empty((NCORES, 128, NBLK), np.float32)
    cw3 = cwg.reshape(NCORES, CT)
    boff = 0
    for j in range(EPC):
        csw[:, :, boff:boff + caps[j]] = (
            cw3[:, ROFF[j]:ROFF[j + 1]]
            .reshape(NCORES, caps[j], 128).transpose(0, 2, 1))
        boff += caps[j]

    has_bias = bool(np.any(be))
    feeds = {"cw": csw.reshape(NCORES * 128, NBLK), "xd": xd, "wd": wd}
    if has_bias:
        feeds["be"] = np.ascontiguousarray(
            be[eid].reshape(NCORES, EPC * NOUT))   # [core][1, e*n] rows
    return caps, has_bias, feeds, dest, inv, aux


def kernel(x, Wg, bg, We, be, k):
    x = np.ascontiguousarray(np.asarray(x, dtype=np.float32))
    Wg = np.asarray(Wg, dtype=np.float32)
    bg = np.asarray(bg, dtype=np.float32)
    We = np.ascontiguousarray(np.asarray(We, dtype=np.float32))
    be = np.ascontiguousarray(np.asarray(be, dtype=np.float32))
    k = int(k)

    caps, has_bias, feeds, dest, inv, aux = _prepare(x, Wg, bg, We, be, k,
                                                     MODE)
    _, run = _get_runner(caps, MODE, has_bias)
    results = run(feeds)

    Y = results["out"]                                  # [E*C, NOUT] global
    slot_rows = Y[dest[inv]]                            # token-major slots
    out = slot_rows.reshape(B, k, NOUT).sum(axis=1, dtype=np.float32)
    return out, aux


# revision 55
# speedup vs baseline: 2.0169x; 1.0493x over previous
"""MoE (top-2 of 32 experts, 512->512) on 8 NeuronCores, expert-parallel.

Strategy (full-I/O contract: kernel() receives full inputs, returns full output):
  - Host computes the small selector (softmax gate + top-k) in fp32 numpy,
    mirroring the reference op-for-op, and performs the "all-to-all dispatch":
    tokens are grouped by expert into capacity-padded batches (the host-side
    sharding step of the expert-parallel layout).
  - Experts are sharded 4-per-core across the 8 cores.  Each core runs a Bass
    kernel: for each of its experts, Y = relu(X_e @ We[e] + be[e]) * gate_w,
    with the per-token gate weight fused into the relu via the ScalarE
    activation (scale is per-partition; gate weights are >= 0 so
    w*relu(z) == relu(w*z)).
  - Host combines: out[token] = sum of its k slot rows (weighted on device).

Precision modes for the expert GEMMs (MODE):
  - "f32"  : exact fp32 matmuls (PE runs them as 2 half-rate passes).
  - "f16x3": x and W split into fp16 hi+lo; y = xh@wh + xh@wl + xl@wh.
             fp16 products are exact in fp32 (11+11 mantissa bits < 24), so
             the only extra error is the dropped xl@wl term (~2^-22 relative)
             - fp32-equivalent accuracy at 3/4 the PE cost of "f32".
  - "f32r" : single-pass relaxed-precision fp32 (tf32-like, ~1e-4 rel err).
  - "bf16" : single-pass bf16 (~1e-3 rel err).

All tensors are pre-swizzled on the host into the exact SBUF layout so every
device DMA is a contiguous copy.  The device kernel is compiled once per
(capacity, mode, has_bias) and cached.
"""

import numpy as np
from contextlib import ExitStack

B, NIN, NOUT, E, NCORES = 8192, 512, 512, 32, 8
EPC = E // NCORES  # experts per core
KCH = NIN // 128   # contraction chunks of 128

MODE = "f16x3"

_CACHE = {}


def _mode_spec(mode):
    import concourse.mybir as mybir
    import ml_dtypes
    if mode == "f32":
        return dict(dt=mybir.dt.float32, npdt=np.float32, nsplit=1,
                    passes=[(0, 0)])
    if mode == "f32r":
        return dict(dt=mybir.dt.float32r, npdt=np.float32, nsplit=1,
                    passes=[(0, 0)])
    if mode == "bf16":
        return dict(dt=mybir.dt.bfloat16, npdt=ml_dtypes.bfloat16, nsplit=1,
                    passes=[(0, 0)])
    if mode == "f16x3":
        return dict(dt=mybir.dt.float16, npdt=np.float16, nsplit=2,
                    passes=[(0, 0), (0, 1), (1, 0)])
    raise ValueError(mode)


def _build(caps, mode, has_bias):
    """Build + compile the per-core Bass program.  caps[j] = number of
    128-token blocks for expert slot j (same structure on every core; the
    host assigns its busiest expert to slot 0 etc. so capacity is not
    wasted padding every expert to the global max)."""
    import concourse.mybir as mybir
    import concourse.tile as tile
    from concourse import bacc

    spec = _mode_spec(mode)
    mmdt = spec["dt"]
    nsplit = spec["nsplit"]
    passes = spec["passes"]
    Cj = list(caps)                        # tokens per slot (multiple of 16)
    CT = sum(Cj)                           # tokens per core
    ROFF = np.concatenate([[0], np.cumsum(Cj)]).astype(int)   # token offsets
    XOFF = [KCH * r for r in ROFF]         # x free-elem offsets per slot
    XF = KCH * CT           # x free elems per split: [p][slot][kc][c]
    WF = EPC * KCH * NOUT   # w free elems per split: [p][slot][kc][n]

    nc = bacc.Bacc("TRN2", target_bir_lowering=False, debug=False,
                   num_devices=NCORES)
    # hi/lo splits are stacked along the free axis of ONE tensor so a single
    # DMA moves both (fewer DMA triggers - they serialize on the HWDGE ring)
    xd = nc.dram_tensor("xd", [128, nsplit * XF], mmdt,
                        kind="ExternalInput").ap()
    wd = nc.dram_tensor("wd", [128, nsplit * WF], mmdt,
                        kind="ExternalInput").ap()
    if has_bias:
        cw = nc.dram_tensor("cw", [1, CT], mybir.dt.float32,
                            kind="ExternalInput").ap()
        be = nc.dram_tensor("be", [1, EPC * NOUT], mybir.dt.float32,
                            kind="ExternalInput").ap()
    # out is transposed: [nout, tokens] (weights are the stationary operand)
    out = nc.dram_tensor("out", [NOUT, CT], mybir.dt.float32,
                         kind="ExternalOutput").ap()

    NS = NOUT // 128                       # nout slices
    # token chunks per slot (moving-N <= 512 per matmul / one PSUM bank)
    def chunks_of(c):
        res, o = [], 0
        while o < c:
            n = min(512, c - o)
            res.append((o, n))
            o += n
        return res

    with tile.TileContext(nc) as tc, ExitStack() as ctx:
        xpool = ctx.enter_context(tc.tile_pool(name="x", bufs=1))
        wpool = ctx.enter_context(tc.tile_pool(name="w", bufs=1))
        spool = ctx.enter_context(tc.tile_pool(name="s", bufs=1))
        opool = ctx.enter_context(tc.tile_pool(name="o", bufs=1))
        pspool = ctx.enter_context(tc.tile_pool(name="ps", bufs=2,
                                                space="PSUM"))

        x_all = xpool.tile([128, nsplit * XF], mmdt, name="x_all")
        w_all = wpool.tile([128, nsplit * WF], mmdt, name="w_all")
        x_sb = [x_all[:, s * XF:(s + 1) * XF] for s in range(nsplit)]
        w_sb = [w_all[:, s * WF:(s + 1) * WF] for s in range(nsplit)]

        if has_bias:
            # tiny and needed by every epilogue: first on the FIFO ring
            cw_sb = spool.tile([1, CT], mybir.dt.float32)
            nc.sync.dma_start(cw_sb[:], cw)
            bias_sb = spool.tile([1, EPC * NOUT], mybir.dt.float32)
            nc.sync.dma_start(bias_sb[:], be)

        xv = xd.rearrange("p (s f) -> p s f", s=nsplit)
        wv = wd.rearrange("p (s f) -> p s f", s=nsplit)
        xav = x_all[:].rearrange("p (s f) -> p s f", s=nsplit)
        wav = w_all[:].rearrange("p (s f) -> p s f", s=nsplit)
        for j in range(EPC):
            if caps[j] == 0:
                continue
            # one DMA covers hi+lo; finer chunks for slot 0 so the first
            # matmul starts sooner.  Slot 0's first k-chunk additionally
            # ships hi before lo, in the order the (kc, pass) matmul stream
            # consumes it: wh, xh (pass 1), wl (pass 2), xl (pass 3).
            if j == 0 and nsplit == 2:
                for (s, a_, v_, l_, r_) in (
                        (0, wav, wv, 0, NOUT),
                        (0, xav, xv, XOFF[0], XOFF[0] + Cj[0]),
                        (1, wav, wv, 0, NOUT),
                        (1, xav, xv, XOFF[0], XOFF[0] + Cj[0])):
                    nc.sync.dma_start(a_[:, s:s + 1, l_:r_],
                                      v_[:, s:s + 1, l_:r_])
            KH = 1 if j == 0 else 2
            for kh in range(KH if j == 0 else 0, KCH, KH):
                xl_ = XOFF[j] + kh * Cj[j]
                xr_ = XOFF[j] + (kh + KH) * Cj[j]
                wl_ = (j * KCH + kh) * NOUT
                wr_ = (j * KCH + kh + KH) * NOUT
                nc.sync.dma_start(xav[:, :, xl_:xr_], xv[:, :, xl_:xr_])
                nc.sync.dma_start(wav[:, :, wl_:wr_], wv[:, :, wl_:wr_])

        # out staging: [p (nout within slice)][ns][token]
        out_sb = opool.tile([128, NS * CT], mybir.dt.float32)

        # PE warmup: dependency-free dummy matmuls ramp the PE clock to full
        # speed while the first input DMAs are still in flight.
        wu_sb = spool.tile([128, 512], mmdt)
        nc.vector.memset(wu_sb[:], 0.0)
        wu_ps = pspool.tile([128, 512], mybir.dt.float32, tag="ps3")
        for _ in range(4):
            nc.tensor.matmul(wu_ps[:], lhsT=wu_sb[:, :128], rhs=wu_sb[:],
                             start=True, stop=True)

        npass = len(passes)
        nmm = KCH * npass + (1 if has_bias else 0)
        for j in range(EPC):
            if caps[j] == 0:
                continue
            for ci, (tok0, tokn) in enumerate(chunks_of(Cj[j])):
                pss = [pspool.tile([128, tokn], mybir.dt.float32,
                                   name=f"ps_{j}_{ci}_{ns}", tag=f"ps{ns}")
                       for ns in range(NS)]
                # k-chunk outer so the first matmuls need only k-chunk 0;
                # slot 0 goes pass-mid to match its hi-first DMA order
                if j == 0:
                    order = [(kc, p, ns) for kc in range(KCH)
                             for p in range(npass) for ns in range(NS)]
                else:
                    order = [(kc, p, ns) for kc in range(KCH)
                             for ns in range(NS) for p in range(npass)]
                for (kc, p, ns) in order:
                    sx, sw = passes[p]
                    m = kc * npass + p + 1
                    nc.tensor.matmul(
                        pss[ns][:],
                        lhsT=w_sb[sw][:, (j * KCH + kc) * NOUT + ns * 128:
                                      (j * KCH + kc) * NOUT + (ns + 1) * 128],
                        rhs=x_sb[sx][:, XOFF[j] + kc * Cj[j] + tok0:
                                     XOFF[j] + kc * Cj[j] + tok0 + tokn],
                        start=(m == 1), stop=(m == nmm))
                last = j == EPC - 1 and tok0 + tokn == Cj[j]
                out_eng = nc.scalar if j < EPC // 2 else nc.sync
                for ns in range(NS):
                    if has_bias:
                        # psum holds w*(x@W); add w*be via rank-1 matmul
                        nc.tensor.matmul(
                            pss[ns][:],
                            lhsT=bias_sb[:, j * NOUT + ns * 128:
                                         j * NOUT + (ns + 1) * 128],
                            rhs=cw_sb[:, ROFF[j] + tok0:
                                      ROFF[j] + tok0 + tokn],
                            start=False, stop=True)
                    off = ns * CT + ROFF[j] + tok0
                    nc.scalar.activation(
                        out_sb[:, off:off + tokn], pss[ns][:],
                        mybir.ActivationFunctionType.Relu)
                    (nc.sync if last else out_eng).dma_start(
                        out[ns * 128:(ns + 1) * 128,
                            ROFF[j] + tok0:ROFF[j] + tok0 + tokn],
                        out_sb[:, off:off + tokn])

    nc.compile()
    return nc


def _make_runner(nc):
    """One-time jit of the 8-core SPMD executable (mirrors
    bass2jax.run_bass_via_pjrt, cached so repeat calls skip retracing)."""
    import jax
    import jax.core
    import numpy as _np
    from jax.sharding import Mesh, PartitionSpec
    from jax.experimental.shard_map import shard_map
    from concourse import bass2jax, mybir

    bass2jax.install_neuronx_cc_hook()

    partition_name = (nc.partition_id_tensor.name
                      if nc.partition_id_tensor else None)
    in_names, out_names, out_avals, zero_shapes = [], [], [], []
    for alloc in nc.m.functions[0].allocations:
        if not isinstance(alloc, mybir.MemoryLocationSet):
            continue
        name = alloc.memorylocations[0].name
        if alloc.kind == "ExternalInput":
            if name != partition_name:
                in_names.append(name)
        elif alloc.kind == "ExternalOutput":
            out_names.append(name)
            shape = tuple(alloc.tensor_shape)
            dt = mybir.dt.np(alloc.dtype)
            out_avals.append(jax.core.ShapedArray(shape, dt))
            zero_shapes.append((shape, dt))
    n_params = len(in_names)
    all_names = in_names + out_names
    if partition_name is not None:
        all_names = all_names + [partition_name]

    def _body(*args):
        operands = list(args)
        if partition_name is not None:
            operands.append(bass2jax.partition_id_tensor())
        outs = bass2jax._bass_exec_p.bind(
            *operands,
            out_avals=tuple(out_avals),
            in_names=tuple(all_names),
            out_names=tuple(out_names),
            lowering_input_output_aliases=(),
            sim_require_finite=True,
            sim_require_nnan=True,
            nc=nc,
        )
        return tuple(outs)

    devices = jax.devices()[:NCORES]
    mesh = Mesh(_np.asarray(devices), ("core",))
    n_outs = len(out_names)
    specs = (PartitionSpec("core"),) * (n_params + n_outs)
    donate = tuple(range(n_params, n_params + n_outs))
    sharded = jax.jit(
        shard_map(_body, mesh=mesh, in_specs=specs,
                  out_specs=(PartitionSpec("core"),) * n_outs,
                  check_rep=False),
        donate_argnums=donate, keep_unused=True)

    def run(feeds):
        """feeds: dict name -> full concatenated array [NCORES*dim0, ...].
        Returns dict name -> full concatenated output array."""
        concat_in = [feeds[name] for name in in_names]
        concat_zeros = [
            _np.zeros((NCORES * s[0],) + tuple(s[1:]), dt)
            for (s, dt) in zero_shapes
        ]
        out_arrs = sharded(*concat_in, *concat_zeros)
        return {name: _np.asarray(out_arrs[i])
                for i, name in enumerate(out_names)}

    # exposed for benchmarking (test.py)
    run._sharded = sharded
    run._in_names = in_names
    run._zero_shapes = zero_shapes
    return run


def _get_runner(caps, mode, has_bias):
    key = (caps, mode, has_bias)
    if key not in _CACHE:
        nc = _build(caps, mode, has_bias)
        _CACHE[key] = (nc, _make_runner(nc))
    return _CACHE[key]


def _route(x, Wg, bg, k):
    """Replicates the reference selector in fp32: softmax gate, top-k
    (stable, ties to lower index like jax.lax.top_k), aux loss."""
    logits = x @ Wg + bg
    m = logits.max(-1, keepdims=True)
    p = np.exp(logits - m)
    gate = p / p.sum(-1, keepdims=True)
    idx = np.argsort(-gate, axis=-1, kind="stable")[:, :k]      # [B, k]
    vals = np.take_along_axis(gate, idx, axis=-1)               # [B, k]
    row_sum = gate.sum(-1)
    aux = (np.var(row_sum) / (np.mean(row_sum) ** 2 + np.float32(1e-10)))
    return idx, vals, np.float32(aux)


def _split_into(dst, view, spec, F):
    """Write `view` [NCORES, 128, F] (fp32, any strides) into dst
    [NCORES*128, nsplit*F] as hi (and lo residual for split modes)."""
    d = dst.reshape(NCORES, 128, -1)
    d[:, :, :F] = view                      # cast fp32 -> device dtype
    if spec["nsplit"] == 2:
        d[:, :, F:] = view - d[:, :, :F].astype(np.float32)


def _prepare(x, Wg, bg, We, be, k, mode):
    """Route + dispatch: returns (caps, has_bias, feeds, dest, inv, aux)."""
    spec = _mode_spec(mode)
    idx, vals, aux = _route(x, Wg, bg, k)

    ef = idx.ravel()
    wf = vals.ravel()
    tf = np.repeat(np.arange(B), k)
    order = np.argsort(ef, kind="stable")
    counts = np.bincount(ef, minlength=E)

    # Load-sorted slot assignment: expert with load-rank r goes to core r%8,
    # slot r//8; slot j's capacity is the max token count in rank octile j
    # (rounded to 16), so every core compiles to the same block structure.
    rank_order = np.argsort(-counts, kind="stable")      # expert ids by load
    caps = tuple(int(max(16, -(-counts[rank_order[NCORES * j]] // 16) * 16))
                 for j in range(EPC))
    Cj = np.array(caps)
    CT = int(Cj.sum())
    ROFF = np.concatenate([[0], np.cumsum(Cj)]).astype(np.int64)

    core_of = np.empty(E, dtype=np.int64)
    slot_of = np.empty(E, dtype=np.int64)
    core_of[rank_order] = np.arange(E) % NCORES
    slot_of[rank_order] = np.arange(E) // NCORES

    starts = np.zeros(E, dtype=np.int64)
    starts[1:] = np.cumsum(counts)[:-1]
    base = core_of * CT + ROFF[slot_of]                  # per-expert row base
    es = ef[order]
    dest = base[es] + (np.arange(B * k) - starts[es])    # unique global rows
    inv = np.argsort(order, kind="stable")

    # Tokens are PRE-SCALED by their gate weight (w >= 0 so
    # relu((w*x)@W + w*be) == w*relu(x@W + be)); padding rows stay zero.
    Xg = np.zeros((NCORES * CT, NIN), dtype=np.float32)
    Xg[dest] = x[tf[order]] * wf[order][:, None]
    cwg = np.zeros(NCORES * CT, dtype=np.float32)
    cwg[dest] = wf[order]

    npdt = spec["npdt"]
    ns = spec["nsplit"]
    XF = KCH * CT
    WF = EPC * KCH * NOUT

    # swizzle straight into the concat-ready device feeds
    # x: [core][p (nin within chunk)][slot][kc][token], token-granular slots
    xsw = np.empty((NCORES, 128, XF), np.float32)
    Xg3 = Xg.reshape(NCORES, CT, NIN)
    for j in range(EPC):
        xsw[:, :, KCH * ROFF[j]:KCH * ROFF[j + 1]] = (
            Xg3[:, ROFF[j]:ROFF[j + 1], :]
            .reshape(NCORES, Cj[j], KCH, 128).transpose(0, 3, 2, 1)
            .reshape(NCORES, 128, KCH * Cj[j]))
    xd = np.empty((NCORES * 128, ns * XF), npdt)
    _split_into(xd, xsw, spec, XF)

    # w: [core][p][slot][kc][n]; core c's slot j holds expert rank_order[8j+c]
    eid = rank_order.reshape(EPC, NCORES).T              # [core, slot]
    wd = np.empty((NCORES * 128, ns * WF), npdt)
    _split_into(wd, We[eid].reshape(NCORES, EPC, KCH, 128, NOUT)
                .transpose(0, 3, 1, 2, 4).reshape(NCORES, 128, WF), spec, WF)

    has_bias = bool(np.any(be))
    feeds = {"xd": xd, "wd": wd}
    if has_bias:
        feeds["cw"] = np.ascontiguousarray(cwg.reshape(NCORES, CT))
        feeds["be"] = np.ascontiguousarray(
            be[eid].reshape(NCORES, EPC * NOUT))   # [core][1, e*n] rows
    return caps, has_bias, feeds, dest, inv, aux, CT


def kernel(x, Wg, bg, We, be, k):
    x = np.ascontiguousarray(np.asarray(x, dtype=np.float32))
    Wg = np.asarray(Wg, dtype=np.float32)
    bg = np.asarray(bg, dtype=np.float32)
    We = np.ascontiguousarray(np.asarray(We, dtype=np.float32))
    be = np.ascontiguousarray(np.asarray(be, dtype=np.float32))
    k = int(k)

    caps, has_bias, feeds, dest, inv, aux, CT = _prepare(x, Wg, bg, We, be,
                                                        k, MODE)
    _, run = _get_runner(caps, MODE, has_bias)
    results = run(feeds)

    # out is [NCORES*NOUT, CT] with nout on rows (weights were stationary)
    Y3 = results["out"].reshape(NCORES, NOUT, CT)
    d = dest[inv]                                       # token-major slots
    slot_rows = Y3[d // CT, :, d % CT]                  # [B*k, NOUT]
    out = slot_rows.reshape(B, k, NOUT).sum(axis=1, dtype=np.float32)
    return out, aux
